# revision 12
# baseline (speedup 1.0000x reference)
"""Trainium2 Bass kernel for the 2-layer hetero-GCN + linear edge decoder.

Math restructuring (exact, up to fp reassociation):
  hetero_conv: out = sum_r nd_r*(A_r @ (ns_r*x)) @ W_r + sum_r b_r
    -> per-edge weight w_e = ns_r[src]*nd_r[dst] folded into a one-hot
       scatter matrix S; aggregation z_r^T = g^T @ S runs directly
       transposed on the TensorEngine (no separate transpose step);
       the W_r matmul happens once per dst-tile.
  decoder has NO nonlinearity between Wp1 and Wp2, so
    score[e] = u[src_e] + v[dst_e] + c,  u = feat @ (Wp1@Wp2)[:256],
    v = feat @ (Wp1@Wp2)[256:], folded into the layer-B weight matmul
    (m = h @ WMcat, 16 useful cols per relation).

All matmuls and gathered tables run in bf16 (4x PE throughput vs fp32,
half the gather bytes); PSUM accumulates fp32.  Gathers are merged per
dst-tile: layer A uses 8 gathers/tile (one per relation-pair x half),
layer B 2 gathers/tile, decoder 1 gather/block.  Trailing slots of the
last relation block in each gather are padded with idx=-1 (skipped by
the DMA engine); the true valid count is loaded per-core into a
register via value_load, keeping the program SPMD-uniform.

Sharding: dst-nodes partitioned into 128-node tiles, 49 tiles/core x 8
cores; x tables replicated; AllGather of the per-node message table m
([npad,128] bf16) and of the tiny (u,v) table; decoder edges sharded
evenly.
"""
import os
import sys

for _p in ("/opt/trn_rl_repo", "/root/.axon_site/_ro/trn_rl_repo"):
    if os.path.isdir(_p) and _p not in sys.path:
        sys.path.append(_p)

import numpy as np
import ml_dtypes

import concourse.bass as bass
import concourse.bacc as bacc
import concourse.mybir as mybir
import concourse.tile as tile
from concourse.bass_utils import run_bass_kernel_spmd
from concourse.masks import make_identity

P = 128
NC = 8
R = 8
F32 = mybir.dt.float32
BF16 = mybir.dt.bfloat16
I16 = mybir.dt.int16
I32 = mybir.dt.int32

DYN_COUNT = os.environ.get("K_DYN_COUNT", "0") == "1"

NPAD = 50176          # 392 tiles of 128
TPC = NPAD // (P * NC)  # 49 tiles per core
NT = NPAD // P        # 392
HALF = NPAD // 2      # 25088, split point for int16 gather indices
XC = 640              # padded x row (bf16 -> 1280B, mult of 256B)
NSL = 5               # 640 / 128 k-slices
D2 = 256
DB = 4096             # decoder edges per block


def _deg_norm(idx, n):
    deg = np.bincount(idx, minlength=n).astype(np.float32)
    out = np.zeros(n, np.float32)
    nz = deg > 0
    out[nz] = 1.0 / np.sqrt(np.maximum(deg[nz], 1.0))
    return out


def _wrap16(a):
    # [G, L] -> [16, G*L/16]: idx i of each group -> [i%16, group*L/16 + i//16]
    G, L = a.shape
    return a.reshape(G, L // 16, 16).transpose(2, 0, 1).reshape(16, G * (L // 16))


def _rep128(a16):
    return np.ascontiguousarray(np.tile(a16, (8, 1)))


def _build(dims):
    CH = dims["CH"]
    SLOT = CH * P
    NCHT = 16 * CH              # chunks per tile (8 rel x 2 halves)
    nb = dims["nb"]
    tpc = TPC
    ac = 2 * SLOT // 16         # idx cols per layer-A gather (pair)
    bc = 8 * SLOT // 16         # idx cols per layer-B gather (region)

    nc = bacc.Bacc("TRN2", target_bir_lowering=False, debug=False)

    xA = nc.declare_dram_parameter("xA", [HALF, XC], BF16, isOutput=False)
    xB = nc.declare_dram_parameter("xB", [NPAD - HALF, XC], BF16, isOutput=False)
    WA = nc.declare_dram_parameter("WA", [R * NSL, P, 256], BF16, isOutput=False)
    WM = nc.declare_dram_parameter("WM", [4, P, P], BF16, isOutput=False)
    bias_rep = nc.declare_dram_parameter("bias_rep", [P, 512], F32, isOutput=False)
    uvb_rep = nc.declare_dram_parameter("uvb_rep", [P, 16], F32, isOutput=False)
    idx_a = nc.declare_dram_parameter("idx_a", [P, tpc * 8 * ac], I16, isOutput=False)
    idx_b = nc.declare_dram_parameter("idx_b", [P, tpc * 2 * bc], I16, isOutput=False)
    smat = nc.declare_dram_parameter(
        "smat", [tpc * NCHT, P, P], BF16, isOutput=False)
    idx_d = nc.declare_dram_parameter("idx_d", [P, nb * 512], I16, isOutput=False)
    msk_d = nc.declare_dram_parameter("msk_d", [P, nb * 8 * 256], F32, isOutput=False)
    score_out = nc.declare_dram_parameter("score_out", [nb * DB, 8], F32, isOutput=True)

    AL = mybir.AluOpType

    with tile.TileContext(nc) as tc:
        with (
            tc.tile_pool(name="cpool", bufs=1) as cp,
            tc.tile_pool(name="dram", bufs=1, space="DRAM") as dp,
        ):
            m_shard = dp.tile([tpc * P, P], BF16)
            m_full = dp.tile([NPAD, P], BF16, addr_space="Shared")
            uv_shard = dp.tile([tpc * P, 16], F32)
            uv_full = dp.tile([NPAD // 4, 64], F32, addr_space="Shared")

            ident = cp.tile([P, P], BF16)
            make_identity(nc, ident[:])
            bias_t = cp.tile([P, 512], F32)
            nc.sync.dma_start(out=bias_t[:], in_=bias_rep[:, :])
            uvb_t = cp.tile([P, 16], F32)
            nc.sync.dma_start(out=uvb_t[:], in_=uvb_rep[:, :])
            wa_t = []
            for i in range(R * NSL):
                w_i = cp.tile([P, 256], BF16, tag=f"wa{i}")
                nc.sync.dma_start(out=w_i[:], in_=WA[i, :, :])
                wa_t.append(w_i)
            wm_t = []
            for k in range(4):
                w_k = cp.tile([P, P], BF16, tag=f"wm{k}")
                nc.sync.dma_start(out=w_k[:], in_=WM[k, :, :])
                wm_t.append(w_k)

            # ---------------- layer A ----------------
            GBUFS = 3
            with (
                tc.tile_pool(name="mpool", bufs=2) as mp,
                tc.tile_pool(name="gpool", bufs=GBUFS) as ga,
                tc.tile_pool(name="spool", bufs=4 * dims["CH"]) as sp,
                tc.tile_pool(name="zpool", bufs=3) as zp,
                tc.tile_pool(name="pszpool", bufs=2, space="PSUM") as psz,
                tc.tile_pool(name="pshpool", bufs=1, space="PSUM") as psh,
                tc.tile_pool(name="pstpool", bufs=2, space="PSUM") as pst,
                tc.tile_pool(name="psmpool", bufs=1, space="PSUM") as psm,
            ):
                for t in range(tpc):
                    ia_t = mp.tile([P, 8 * ac], I16, tag="ia")
                    nc.sync.dma_start(out=ia_t[:], in_=idx_a[:, t * 8 * ac:(t + 1) * 8 * ac])
                    gt = []
                    for h in (0, 1):   # one merged gather per half (4 pairs)
                        g = ga.tile([P, 8 * CH, XC], BF16, tag="g")
                        nc.gpsimd.dma_gather(
                            out_ap=g[:], in_ap=(xA if h == 0 else xB)[:, :],
                            idxs_ap=ia_t[:, h * 4 * ac:(h + 1) * 4 * ac],
                            num_idxs=8 * SLOT, num_idxs_reg=8 * SLOT,
                            elem_size=XC, single_packet=False)
                        gt.append(g)
                    h2 = psh.tile([P, 256], F32, tag="h2")
                    h3 = psh.tile([P, 256], F32, tag="h3")
                    for r in range(R):
                        seq = []
                        for h in (0, 1):
                            for cc in range(CH):
                                lc = (r // 2) * 2 * CH + (r % 2) * CH + cc
                                seq.append((gt[h], lc, (h * 8 + r) * CH + cc))
                        s_tiles = []
                        for (gtile, lc, gch) in seq:
                            s = sp.tile([P, P], BF16, tag="s")
                            nc.sync.dma_start(
                                out=s[:], in_=smat[t * NCHT + gch, :, :])
                            s_tiles.append(s)
                        for k in range(NSL):
                            ztk = psz.tile([P, P], F32, tag="ztk")
                            for i, (gtile, lc, gch) in enumerate(seq):
                                nc.tensor.matmul(
                                    ztk[:], gtile[:, lc, k * P:(k + 1) * P],
                                    s_tiles[i][:],
                                    start=(i == 0), stop=(i == 2 * CH - 1))
                            zsk = zp.tile([P, P], BF16, tag="zsk")
                            nc.scalar.copy(out=zsk[:], in_=ztk[:])
                            nc.tensor.matmul(
                                (h2 if k < 2 else h3)[:],
                                zsk[:], wa_t[r * NSL + k][:],
                                start=(r == 0 and k in (0, 2)),
                                stop=(r == R - 1 and k in (1, NSL - 1)))
                    hsb = zp.tile([P, 512], BF16, tag="hsb")
                    nc.vector.tensor_tensor(out=hsb[:, 0:256], in0=h2[:],
                                            in1=bias_t[:, 0:256], op=AL.add)
                    nc.vector.tensor_tensor(out=hsb[:, 256:512], in0=h3[:],
                                            in1=bias_t[:, 256:512], op=AL.add)
                    nc.vector.tensor_scalar_max(out=hsb[:], in0=hsb[:], scalar1=0.0)
                    ht4 = zp.tile([P, 4, P], BF16, tag="ht4")
                    for k in range(4):
                        tp = pst.tile([P, P], BF16, tag="tp")
                        nc.tensor.transpose(out=tp[:], in_=hsb[:, k * P:(k + 1) * P],
                                            identity=ident[:])
                        nc.scalar.copy(out=ht4[:, k, :], in_=tp[:])
                    mps = psm.tile([P, P], F32, tag="mps")
                    for k in range(4):
                        nc.tensor.matmul(mps[:], ht4[:, k, :], wm_t[k][:],
                                         start=(k == 0), stop=(k == 3))
                    msb = zp.tile([P, P], BF16, tag="msb")
                    nc.vector.tensor_copy(out=msb[:], in_=mps[:])
                    nc.sync.dma_start(out=m_shard[t * P:(t + 1) * P, :], in_=msb[:])

            nc.gpsimd.collective_compute(
                "AllGather", mybir.AluOpType.bypass,
                replica_groups=[list(range(NC))],
                ins=[m_shard[:, :]], outs=[m_full[:, :]])

            # ---------------- layer B ----------------
            GBBUFS = 4
            with (
                tc.tile_pool(name="mpoolb", bufs=2) as mp,
                tc.tile_pool(name="gpoolb", bufs=GBBUFS) as gb_p,
                tc.tile_pool(name="spoolb", bufs=2 * dims["CH"] * 2) as sp,
                tc.tile_pool(name="zpoolb", bufs=2) as zp,
                tc.tile_pool(name="psupool", bufs=2, space="PSUM") as psu,
            ):
                for t in range(tpc):
                    ib_t = mp.tile([P, 2 * bc], I16, tag="ib")
                    nc.sync.dma_start(out=ib_t[:], in_=idx_b[:, t * 2 * bc:(t + 1) * 2 * bc])
                    gbt = []
                    for h in (0, 1):
                        g = gb_p.tile([P, 8 * CH, P], BF16, tag="gb")
                        nc.gpsimd.dma_gather(
                            out_ap=g[:],
                            in_ap=(m_full[0:HALF, :] if h == 0 else m_full[HALF:NPAD, :]),
                            idxs_ap=ib_t[:, h * bc:(h + 1) * bc],
                            num_idxs=8 * SLOT, num_idxs_reg=8 * SLOT, elem_size=P,
                            single_packet=False)
                        gbt.append(g)
                    uvp = psu.tile([P, 16], F32, tag="uvp")
                    for h in (0, 1):
                        for r in range(R):
                            for cc in range(CH):
                                gch = (h * 8 + r) * CH + cc
                                s = sp.tile([P, P], BF16, tag="sb")
                                nc.sync.dma_start(
                                    out=s[:], in_=smat[t * NCHT + gch, :, :])
                                nc.tensor.matmul(
                                    uvp[:], s[:],
                                    gbt[h][:, r * CH + cc, r * 16:(r + 1) * 16],
                                    start=(gch == 0), stop=(gch == NCHT - 1))
                    uvsb = zp.tile([P, 16], F32, tag="uvsb")
                    nc.vector.tensor_tensor(out=uvsb[:], in0=uvp[:], in1=uvb_t[:],
                                            op=AL.add)
                    nc.sync.dma_start(out=uv_shard[t * P:(t + 1) * P, :], in_=uvsb[:])

            nc.gpsimd.collective_compute(
                "AllGather", mybir.AluOpType.bypass,
                replica_groups=[list(range(NC))],
                ins=[uv_shard[:, :]], outs=[uv_full[:, :]])

            # ---------------- decoder ----------------
            with (
                tc.tile_pool(name="dmp", bufs=2) as mp,
                tc.tile_pool(name="dgp", bufs=2) as gp,
                tc.tile_pool(name="dvp", bufs=2) as vp,
            ):
                sview = score_out.ap().rearrange("(B j p) d -> B p j d", p=P, j=32)
                for b in range(nb):
                    id_t = mp.tile([P, 512], I16, tag="id")
                    nc.sync.dma_start(out=id_t[:], in_=idx_d[:, b * 512:(b + 1) * 512])
                    mk_t = mp.tile([P, 8, 32, 8], F32, tag="mk")
                    nc.sync.dma_start(
                        out=mk_t[:],
                        in_=msk_d[:, b * 2048:(b + 1) * 2048])
                    gd = gp.tile([P, 64, 64], F32, tag="gd")
                    nc.gpsimd.dma_gather(
                        out_ap=gd[:, 0:32, :], in_ap=uv_full[:, :],
                        idxs_ap=id_t[:, 0:256],
                        num_idxs=4096, num_idxs_reg=4096, elem_size=64,
                        single_packet=False)
                    nc.gpsimd.dma_gather(
                        out_ap=gd[:, 32:64, :], in_ap=uv_full[:, :],
                        idxs_ap=id_t[:, 256:512],
                        num_idxs=4096, num_idxs_reg=4096, elem_size=64,
                        single_packet=False)
                    acc = vp.tile([P, 32, 8], F32, tag="acc")
                    accv = vp.tile([P, 32, 8], F32, tag="accv")
                    for b4 in range(4):
                        if b4 == 0:
                            nc.vector.tensor_tensor(
                                out=acc[:], in0=mk_t[:, 0, :, :],
                                in1=gd[:, 0:32, 0:8], op=AL.mult)
                        else:
                            eq = vp.tile([P, 32, 8], F32, tag="eq")
                            nc.vector.tensor_tensor(
                                out=eq[:], in0=mk_t[:, b4, :, :],
                                in1=gd[:, 0:32, b4 * 16:b4 * 16 + 8], op=AL.mult)
                            nc.vector.tensor_tensor(out=acc[:], in0=acc[:], in1=eq[:],
                                                    op=AL.add)
                    for b4 in range(4):
                        if b4 == 0:
                            nc.vector.tensor_tensor(
                                out=accv[:], in0=mk_t[:, 4, :, :],
                                in1=gd[:, 32:64, 8:16], op=AL.mult)
                        else:
                            eq = vp.tile([P, 32, 8], F32, tag="eq")
                            nc.vector.tensor_tensor(
                                out=eq[:], in0=mk_t[:, 4 + b4, :, :],
                                in1=gd[:, 32:64, b4 * 16 + 8:b4 * 16 + 16], op=AL.mult)
                            nc.vector.tensor_tensor(out=accv[:], in0=accv[:], in1=eq[:],
                                                    op=AL.add)
                    nc.vector.tensor_tensor(out=acc[:], in0=acc[:], in1=accv[:],
                                            op=AL.add)
                    nc.sync.dma_start(out=sview[b], in_=acc[:])
    nc.finalize()
    return nc


def _prep(inputs):
    x2 = np.asarray(inputs["node2_features"], np.float32)
    x3 = np.asarray(inputs["mpnn_features"], np.float32)
    src = np.asarray(inputs["src"]).astype(np.int64)
    dst = np.asarray(inputs["dst"]).astype(np.int64)
    dec_src = np.asarray(inputs["dec_src"]).astype(np.int64)
    dec_dst = np.asarray(inputs["dec_dst"]).astype(np.int64)
    W2a = np.asarray(inputs["W2a"], np.float32)
    b2a = np.asarray(inputs["b2a"], np.float32)
    W2b = np.asarray(inputs["W2b"], np.float32)
    b2b = np.asarray(inputs["b2b"], np.float32)
    W3a = np.asarray(inputs["W3a"], np.float32)
    b3a = np.asarray(inputs["b3a"], np.float32)
    W3b = np.asarray(inputs["W3b"], np.float32)
    b3b = np.asarray(inputs["b3b"], np.float32)
    Wp1 = np.asarray(inputs["Wp1"], np.float32)
    bp1 = np.asarray(inputs["bp1"], np.float32)
    Wp2 = np.asarray(inputs["Wp2"], np.float32)
    bp2 = np.asarray(inputs["bp2"], np.float32)

    n = x2.shape[0]
    assert n == 50000 and src.shape == (R, 200000)
    ed = dec_src.shape[0]
    d3 = x3.shape[1]

    # per-edge weights
    ns_arr = np.stack([_deg_norm(src[r], n) for r in range(R)])
    nd_arr = np.stack([_deg_norm(dst[r], n) for r in range(R)])
    src_f = src.ravel()
    dst_f = dst.ravel()
    rel_f = np.repeat(np.arange(R, dtype=np.int64), src.shape[1])
    w_f = (ns_arr[rel_f, src_f] * nd_arr[rel_f, dst_f]).astype(np.float32)
    tile_f = dst_f >> 7
    half_f = (src_f >= HALF).astype(np.int64)

    cnt = np.bincount((tile_f * R + rel_f) * 2 + half_f,
                      minlength=NT * R * 2).reshape(NT, R, 2)
    CH = max(1, -(-int(cnt.max()) // P))
    SLOT = CH * P

    # rank of each edge within its (tile, rel, half) group
    key = (tile_f * R + rel_f) * 2 + half_f
    order = np.lexsort((half_f, rel_f, tile_f))
    ko = key[order]
    starts = np.zeros(NT * R * 16 // 8, np.int64)
    counts_flat = np.bincount(ko, minlength=NT * R * 2)
    starts = np.zeros(NT * R * 2, np.int64)
    np.cumsum(counts_flat[:-1], out=starts[1:])
    rank = np.empty_like(ko)
    rank[:] = np.arange(len(ko)) - starts[ko]
    so, do_, ro, ho, wo = (a[order] for a in (src_f, dst_f, rel_f, half_f, w_f))

    # idx values (relative to half table) and slot assignment
    ival = (so - ho * HALF).astype(np.int64)
    # full idx cube [NT, 2, R, SLOT]; pad pattern applied after filling
    idx_cube = np.zeros((NT, 2, R, SLOT), np.int64)
    idx_cube[tile_f[order], ho, ro, rank] = ival
    valid = np.zeros((NT, 2, R, SLOT), bool)
    valid[tile_f[order], ho, ro, rank] = True

    ia_cube = np.where(valid, idx_cube, 0).astype(np.int16)
    ib_cube = ia_cube

    # host-built one-hot scatter matrices S: [NT, NCHT, 128 slots, 128 dst]
    dloc = np.zeros((NT, 2, R, SLOT), np.int64)
    wval = np.zeros((NT, 2, R, SLOT), np.float32)
    dloc[tile_f[order], ho, ro, rank] = do_ & 127
    wval[tile_f[order], ho, ro, rank] = wo
    SA6 = np.zeros((NT, 2, R, CH, P, P), np.float32)
    np.put_along_axis(
        SA6, dloc.reshape(NT, 2, R, CH, P)[..., None],
        wval.reshape(NT, 2, R, CH, P)[..., None], axis=5)
    smat_all = SA6.reshape(NT, 2 * R * CH, P, P).astype(ml_dtypes.bfloat16)
    del SA6

    # x tables (bf16)
    xcat = np.zeros((NPAD, XC), np.float32)
    xcat[:n, :D2] = x2
    xcat[:n, D2:D2 + d3] = x3
    xcat = xcat.astype(ml_dtypes.bfloat16)
    xA_h = np.ascontiguousarray(xcat[:HALF])
    xB_h = np.ascontiguousarray(xcat[HALF:])

    # layer-A weights [R*NSL, 128, 256]
    WAp = np.zeros((R, NSL, P, 256), np.float32)
    for r in range(R):
        WAp[r, 0] = W2a[r, 0:P]
        WAp[r, 1] = W2a[r, P:2 * P]
        for k in range(2, NSL):
            f0 = k * P - D2
            rows = min(d3 - f0, P) if f0 < d3 else 0
            if rows > 0:
                WAp[r, k, :rows] = W3a[r, f0:f0 + rows]
    WAp = WAp.reshape(R * NSL, P, 256).astype(ml_dtypes.bfloat16)

    # decoder folding
    M = Wp1 @ Wp2
    A2, A3, B2, B3 = M[0:128], M[128:256], M[256:384], M[384:512]
    WMcat = np.zeros((512, P), np.float32)
    for r in range(R):
        WMcat[0:256, r * 16:(r + 1) * 16] = W2b[r] @ np.concatenate([A2, B2], axis=1)
        WMcat[256:512, r * 16:(r + 1) * 16] = W3b[r] @ np.concatenate([A3, B3], axis=1)
    WMp = WMcat.reshape(4, P, P).astype(ml_dtypes.bfloat16)
    c_total = (b2b.sum(0) @ np.concatenate([A2, B2], axis=1)
               + b3b.sum(0) @ np.concatenate([A3, B3], axis=1))
    c_total = c_total[0:8] + c_total[8:16] + bp1 @ Wp2 + bp2
    uvb_rep = np.tile(np.concatenate([np.zeros(8, np.float32),
                                      c_total.astype(np.float32)]), (P, 1))
    bias_rep = np.tile(np.concatenate([b2a.sum(0), b3a.sum(0)]).astype(np.float32), (P, 1))

    # decoder edges
    epc = -(-ed // NC)                       # 50000
    nb = -(-epc // DB)                       # 13
    in_maps = []
    for c in range(NC):
        tsl = slice(c * TPC, (c + 1) * TPC)
        ia_c = _rep128(_wrap16(
            ia_cube[tsl].transpose(0, 1, 2, 3).reshape(TPC, 2, 4, 2 * SLOT)
            .reshape(TPC * 8, 2 * SLOT)))
        ib_c = _rep128(_wrap16(ib_cube[tsl].reshape(TPC * 2, 8 * SLOT)))
        smat_c = np.ascontiguousarray(
            smat_all[tsl].reshape(TPC * 2 * R * CH, P, P))

        e0 = c * epc
        s_pad = np.zeros(nb * DB, np.int64)
        d_pad = np.zeros(nb * DB, np.int64)
        seg = slice(e0, min(e0 + epc, ed))
        ln = seg.stop - seg.start
        s_pad[:ln] = dec_src[seg]
        d_pad[:ln] = dec_dst[seg]
        di = np.concatenate([(s_pad >> 2).reshape(nb, DB),
                             (d_pad >> 2).reshape(nb, DB)], axis=1).astype(np.int16)
        di_c = _rep128(_wrap16(di))          # [128, nb*512]
        mu = (s_pad & 3).astype(np.float32).reshape(nb, 32, P).transpose(0, 2, 1)
        mv = (d_pad & 3).astype(np.float32).reshape(nb, 32, P).transpose(0, 2, 1)
        # [nb, 128, 32] -> repeat 8 -> [nb, 128, 256]
        mu = np.repeat(mu, 8, axis=2)
        mv = np.repeat(mv, 8, axis=2)
        # one-hot select masks per quarter: [nb, 8, 128, 256]
        em = np.stack([(mu == b4).astype(np.float32) for b4 in range(4)]
                      + [(mv == b4).astype(np.float32) for b4 in range(4)], axis=1)
        mk = np.ascontiguousarray(
            em.transpose(2, 0, 1, 3).reshape(P, nb * 8 * 256))
        in_maps.append(dict(
            xA=xA_h, xB=xB_h, WA=WAp, WM=WMp, bias_rep=bias_rep,
            uvb_rep=uvb_rep,
            idx_a=ia_c, idx_b=ib_c, smat=smat_c,
            idx_d=di_c, msk_d=mk,
        ))
    dims = dict(CH=CH, nb=nb, epc=epc, ed=ed)
    return in_maps, dims


_CACHE = {}


def kernel(**inputs):
    in_maps, dims = _prep(inputs)
    key = (dims["CH"], dims["nb"])
    nc = _CACHE.get(key)
    if nc is None:
        nc = _build(dims)
        _CACHE[key] = nc
    res = run_bass_kernel_spmd(nc, in_maps, list(range(NC)))
    epc, ed = dims["epc"], dims["ed"]
    out = np.concatenate(
        [res.results[c]["score_out"][:min(epc, ed - c * epc)] for c in range(NC)], axis=0)
    return np.ascontiguousarray(out.astype(np.float32))


if __name__ == "__main__":
    pass



# revision 20
# speedup vs baseline: 1.4424x; 1.4424x over previous
"""Trainium2 Bass kernel for the 2-layer hetero-GCN + linear edge decoder.

Math restructuring (exact, up to fp reassociation):
  hetero_conv: out = sum_r nd_r*(A_r @ (ns_r*x)) @ W_r + sum_r b_r
    -> per-edge weight w_e = ns_r[src]*nd_r[dst] folded into a one-hot
       scatter matrix S; aggregation z_r^T = g^T @ S runs directly
       transposed on the TensorEngine (no separate transpose step);
       the W_r matmul happens once per dst-tile.
  decoder has NO nonlinearity between Wp1 and Wp2, so
    score[e] = u[src_e] + v[dst_e] + c,  u = feat @ (Wp1@Wp2)[:256],
    v = feat @ (Wp1@Wp2)[256:], folded into the layer-B weight matmul
    (m = h @ WMcat, 16 useful cols per relation).

All matmuls and gathered tables run in bf16 (4x PE throughput vs fp32,
half the gather bytes); PSUM accumulates fp32.  Gathers are merged per
dst-tile: layer A uses 8 gathers/tile (one per relation-pair x half),
layer B 2 gathers/tile, decoder 1 gather/block.  Trailing slots of the
last relation block in each gather are padded with idx=-1 (skipped by
the DMA engine); the true valid count is loaded per-core into a
register via value_load, keeping the program SPMD-uniform.

Sharding: dst-nodes partitioned into 128-node tiles, 49 tiles/core x 8
cores; x tables replicated; AllGather of the per-node message table m
([npad,128] bf16) and of the tiny (u,v) table; decoder edges sharded
evenly.
"""
import os
import sys

for _p in ("/opt/trn_rl_repo", "/root/.axon_site/_ro/trn_rl_repo"):
    if os.path.isdir(_p) and _p not in sys.path:
        sys.path.append(_p)

import numpy as np
import ml_dtypes

import concourse.bass as bass
import concourse.bacc as bacc
import concourse.mybir as mybir
import concourse.tile as tile
from concourse.bass_utils import run_bass_kernel_spmd
from concourse.masks import make_identity

P = 128
NC = 8
R = 8
F32 = mybir.dt.float32
BF16 = mybir.dt.bfloat16
I16 = mybir.dt.int16
I32 = mybir.dt.int32

DYN_COUNT = os.environ.get("K_DYN_COUNT", "0") == "1"

NPAD = 50176          # 392 tiles of 128
TPC = NPAD // (P * NC)  # 49 tiles per core
NT = NPAD // P        # 392
HALF = NPAD // 2      # 25088, split point for int16 gather indices
XC = 640              # padded x row (bf16 -> 1280B, mult of 256B)
NSL = 5               # 640 / 128 k-slices
D2 = 256
DB = 4096             # decoder edges per block


def _deg_norm(idx, n):
    deg = np.bincount(idx, minlength=n).astype(np.float32)
    out = np.zeros(n, np.float32)
    nz = deg > 0
    out[nz] = 1.0 / np.sqrt(np.maximum(deg[nz], 1.0))
    return out


def _wrap16(a):
    # [G, L] -> [16, G*L/16]: idx i of each group -> [i%16, group*L/16 + i//16]
    G, L = a.shape
    return a.reshape(G, L // 16, 16).transpose(2, 0, 1).reshape(16, G * (L // 16))


def _rep128(a16):
    return np.ascontiguousarray(np.tile(a16, (8, 1)))


def _build(dims):
    CH = dims["CH"]
    SLOT = CH * P
    NCHT = 16 * CH              # chunks per tile (8 rel x 2 halves)
    nb = dims["nb"]
    tpc = TPC
    ac = 2 * SLOT // 16         # idx cols per layer-A gather (pair)
    bc = 8 * SLOT // 16         # idx cols per layer-B gather (region)

    nc = bacc.Bacc("TRN2", target_bir_lowering=False, debug=False)

    xA = nc.declare_dram_parameter("xA", [HALF, XC], BF16, isOutput=False)
    xB = nc.declare_dram_parameter("xB", [NPAD - HALF, XC], BF16, isOutput=False)
    WA = nc.declare_dram_parameter("WA", [R * NSL, P, 256], BF16, isOutput=False)
    WM = nc.declare_dram_parameter("WM", [4, P, P], BF16, isOutput=False)
    bias_rep = nc.declare_dram_parameter("bias_rep", [P, 512], F32, isOutput=False)
    uvb_rep = nc.declare_dram_parameter("uvb_rep", [P, 16], F32, isOutput=False)
    idx_a = nc.declare_dram_parameter("idx_a", [P, tpc * 8 * ac], I16, isOutput=False)
    idx_b = nc.declare_dram_parameter("idx_b", [P, tpc * 2 * bc], I16, isOutput=False)
    smat = nc.declare_dram_parameter(
        "smat", [tpc, P, NCHT * P], BF16, isOutput=False)
    idx_d = nc.declare_dram_parameter("idx_d", [P, nb * 512], I16, isOutput=False)
    msk_d = nc.declare_dram_parameter("msk_d", [P, nb * 8 * 256], F32, isOutput=False)
    score_out = nc.declare_dram_parameter("score_out", [nb * DB, 8], F32, isOutput=True)

    AL = mybir.AluOpType

    with tile.TileContext(nc) as tc:
        with (
            tc.tile_pool(name="cpool", bufs=1) as cp,
            tc.tile_pool(name="dram", bufs=1, space="DRAM") as dp,
        ):
            m_shard = dp.tile([tpc * P, P], BF16)
            m_full = dp.tile([NPAD, P], BF16, addr_space="Shared")
            uv_shard = dp.tile([tpc * P, 16], F32)
            uv_full = dp.tile([NPAD // 4, 64], F32, addr_space="Shared")

            ident = cp.tile([P, P], BF16)
            make_identity(nc, ident[:])
            bias_t = cp.tile([P, 512], F32)
            nc.sync.dma_start(out=bias_t[:], in_=bias_rep[:, :])
            uvb_t = cp.tile([P, 16], F32)
            nc.sync.dma_start(out=uvb_t[:], in_=uvb_rep[:, :])
            wa_t = []
            for i in range(R * NSL):
                w_i = cp.tile([P, 256], BF16, tag=f"wa{i}")
                nc.sync.dma_start(out=w_i[:], in_=WA[i, :, :])
                wa_t.append(w_i)
            wm_t = []
            for k in range(4):
                w_k = cp.tile([P, P], BF16, tag=f"wm{k}")
                nc.sync.dma_start(out=w_k[:], in_=WM[k, :, :])
                wm_t.append(w_k)

            # ---------------- layer A ----------------
            GBUFS = 3
            with (
                tc.tile_pool(name="mpool", bufs=2) as mp,
                tc.tile_pool(name="gpool", bufs=GBUFS) as ga,
                tc.tile_pool(name="spool", bufs=2) as sp,
                tc.tile_pool(name="zpool", bufs=3) as zp,
                tc.tile_pool(name="pszpool", bufs=2, space="PSUM") as psz,
                tc.tile_pool(name="pshpool", bufs=1, space="PSUM") as psh,
                tc.tile_pool(name="pstpool", bufs=2, space="PSUM") as pst,
                tc.tile_pool(name="psmpool", bufs=1, space="PSUM") as psm,
            ):
                for t in range(tpc):
                    ia_t = mp.tile([P, 8 * ac], I16, tag="ia")
                    nc.sync.dma_start(out=ia_t[:], in_=idx_a[:, t * 8 * ac:(t + 1) * 8 * ac])
                    s_all = sp.tile([P, NCHT * P], BF16, tag="sall")
                    nc.sync.dma_start(out=s_all[:], in_=smat[t, :, :])
                    gt = []
                    for h in (0, 1):   # one merged gather per half (4 pairs)
                        g = ga.tile([P, 8 * CH, XC], BF16, tag="g")
                        nc.gpsimd.dma_gather(
                            out_ap=g[:], in_ap=(xA if h == 0 else xB)[:, :],
                            idxs_ap=ia_t[:, h * 4 * ac:(h + 1) * 4 * ac],
                            num_idxs=8 * SLOT, num_idxs_reg=8 * SLOT,
                            elem_size=XC, single_packet=False)
                        gt.append(g)
                    h2 = psh.tile([P, 256], F32, tag="h2")
                    h3 = psh.tile([P, 256], F32, tag="h3")
                    for r in range(R):
                        seq = []
                        for h in (0, 1):
                            for cc in range(CH):
                                lc = (r // 2) * 2 * CH + (r % 2) * CH + cc
                                seq.append((gt[h], lc, (h * 8 + r) * CH + cc))
                        for k in range(NSL):
                            ztk = psz.tile([P, P], F32, tag="ztk")
                            for i, (gtile, lc, gch) in enumerate(seq):
                                nc.tensor.matmul(
                                    ztk[:], gtile[:, lc, k * P:(k + 1) * P],
                                    s_all[:, gch * P:(gch + 1) * P],
                                    start=(i == 0), stop=(i == 2 * CH - 1))
                            zsk = zp.tile([P, P], BF16, tag="zsk")
                            nc.scalar.copy(out=zsk[:], in_=ztk[:])
                            nc.tensor.matmul(
                                (h2 if k < 2 else h3)[:],
                                zsk[:], wa_t[r * NSL + k][:],
                                start=(r == 0 and k in (0, 2)),
                                stop=(r == R - 1 and k in (1, NSL - 1)))
                    hsb = zp.tile([P, 512], BF16, tag="hsb")
                    nc.vector.tensor_tensor(out=hsb[:, 0:256], in0=h2[:],
                                            in1=bias_t[:, 0:256], op=AL.add)
                    nc.vector.tensor_tensor(out=hsb[:, 256:512], in0=h3[:],
                                            in1=bias_t[:, 256:512], op=AL.add)
                    nc.vector.tensor_scalar_max(out=hsb[:], in0=hsb[:], scalar1=0.0)
                    ht4 = zp.tile([P, 4, P], BF16, tag="ht4")
                    for k in range(4):
                        tp = pst.tile([P, P], BF16, tag="tp")
                        nc.tensor.transpose(out=tp[:], in_=hsb[:, k * P:(k + 1) * P],
                                            identity=ident[:])
                        nc.scalar.copy(out=ht4[:, k, :], in_=tp[:])
                    mps = psm.tile([P, P], F32, tag="mps")
                    for k in range(4):
                        nc.tensor.matmul(mps[:], ht4[:, k, :], wm_t[k][:],
                                         start=(k == 0), stop=(k == 3))
                    msb = zp.tile([P, P], BF16, tag="msb")
                    nc.vector.tensor_copy(out=msb[:], in_=mps[:])
                    nc.sync.dma_start(out=m_shard[t * P:(t + 1) * P, :], in_=msb[:])

            nc.gpsimd.collective_compute(
                "AllGather", mybir.AluOpType.bypass,
                replica_groups=[list(range(NC))],
                ins=[m_shard[:, :]], outs=[m_full[:, :]])

            # ---------------- layer B ----------------
            GBBUFS = 4
            with (
                tc.tile_pool(name="mpoolb", bufs=2) as mp,
                tc.tile_pool(name="gpoolb", bufs=GBBUFS) as gb_p,
                tc.tile_pool(name="spoolb", bufs=2) as sp,
                tc.tile_pool(name="zpoolb", bufs=2) as zp,
                tc.tile_pool(name="psupool", bufs=2, space="PSUM") as psu,
            ):
                for t in range(tpc):
                    ib_t = mp.tile([P, 2 * bc], I16, tag="ib")
                    nc.sync.dma_start(out=ib_t[:], in_=idx_b[:, t * 2 * bc:(t + 1) * 2 * bc])
                    s_all = sp.tile([P, NCHT * P], BF16, tag="sallb")
                    nc.sync.dma_start(out=s_all[:], in_=smat[t, :, :])
                    gbt = []
                    for h in (0, 1):
                        g = gb_p.tile([P, 8 * CH, P], BF16, tag="gb")
                        nc.gpsimd.dma_gather(
                            out_ap=g[:],
                            in_ap=(m_full[0:HALF, :] if h == 0 else m_full[HALF:NPAD, :]),
                            idxs_ap=ib_t[:, h * bc:(h + 1) * bc],
                            num_idxs=8 * SLOT, num_idxs_reg=8 * SLOT, elem_size=P,
                            single_packet=False)
                        gbt.append(g)
                    uvp = psu.tile([P, 16], F32, tag="uvp")
                    for h in (0, 1):
                        for r in range(R):
                            for cc in range(CH):
                                gch = (h * 8 + r) * CH + cc
                                nc.tensor.matmul(
                                    uvp[:], s_all[:, gch * P:(gch + 1) * P],
                                    gbt[h][:, r * CH + cc, r * 16:(r + 1) * 16],
                                    start=(gch == 0), stop=(gch == NCHT - 1))
                    uvsb = zp.tile([P, 16], F32, tag="uvsb")
                    nc.vector.tensor_tensor(out=uvsb[:], in0=uvp[:], in1=uvb_t[:],
                                            op=AL.add)
                    nc.sync.dma_start(out=uv_shard[t * P:(t + 1) * P, :], in_=uvsb[:])

            nc.gpsimd.collective_compute(
                "AllGather", mybir.AluOpType.bypass,
                replica_groups=[list(range(NC))],
                ins=[uv_shard[:, :]], outs=[uv_full[:, :]])

            # ---------------- decoder ----------------
            with (
                tc.tile_pool(name="dmp", bufs=2) as mp,
                tc.tile_pool(name="dgp", bufs=2) as gp,
                tc.tile_pool(name="dvp", bufs=2) as vp,
            ):
                sview = score_out.ap().rearrange("(B j p) d -> B p j d", p=P, j=32)
                for b in range(nb):
                    id_t = mp.tile([P, 512], I16, tag="id")
                    nc.sync.dma_start(out=id_t[:], in_=idx_d[:, b * 512:(b + 1) * 512])
                    mk_t = mp.tile([P, 8, 32, 8], F32, tag="mk")
                    nc.sync.dma_start(
                        out=mk_t[:],
                        in_=msk_d[:, b * 2048:(b + 1) * 2048])
                    gd = gp.tile([P, 64, 64], F32, tag="gd")
                    nc.gpsimd.dma_gather(
                        out_ap=gd[:, 0:32, :], in_ap=uv_full[:, :],
                        idxs_ap=id_t[:, 0:256],
                        num_idxs=4096, num_idxs_reg=4096, elem_size=64,
                        single_packet=False)
                    nc.gpsimd.dma_gather(
                        out_ap=gd[:, 32:64, :], in_ap=uv_full[:, :],
                        idxs_ap=id_t[:, 256:512],
                        num_idxs=4096, num_idxs_reg=4096, elem_size=64,
                        single_packet=False)
                    acc = vp.tile([P, 32, 8], F32, tag="acc")
                    accv = vp.tile([P, 32, 8], F32, tag="accv")
                    for b4 in range(4):
                        if b4 == 0:
                            nc.vector.tensor_tensor(
                                out=acc[:], in0=mk_t[:, 0, :, :],
                                in1=gd[:, 0:32, 0:8], op=AL.mult)
                        else:
                            eq = vp.tile([P, 32, 8], F32, tag="eq")
                            nc.vector.tensor_tensor(
                                out=eq[:], in0=mk_t[:, b4, :, :],
                                in1=gd[:, 0:32, b4 * 16:b4 * 16 + 8], op=AL.mult)
                            nc.vector.tensor_tensor(out=acc[:], in0=acc[:], in1=eq[:],
                                                    op=AL.add)
                    for b4 in range(4):
                        if b4 == 0:
                            nc.vector.tensor_tensor(
                                out=accv[:], in0=mk_t[:, 4, :, :],
                                in1=gd[:, 32:64, 8:16], op=AL.mult)
                        else:
                            eq = vp.tile([P, 32, 8], F32, tag="eq")
                            nc.vector.tensor_tensor(
                                out=eq[:], in0=mk_t[:, 4 + b4, :, :],
                                in1=gd[:, 32:64, b4 * 16 + 8:b4 * 16 + 16], op=AL.mult)
                            nc.vector.tensor_tensor(out=accv[:], in0=accv[:], in1=eq[:],
                                                    op=AL.add)
                    nc.vector.tensor_tensor(out=acc[:], in0=acc[:], in1=accv[:],
                                            op=AL.add)
                    nc.sync.dma_start(out=sview[b], in_=acc[:])
    nc.finalize()
    return nc


def _prep(inputs):
    x2 = np.asarray(inputs["node2_features"], np.float32)
    x3 = np.asarray(inputs["mpnn_features"], np.float32)
    src = np.asarray(inputs["src"]).astype(np.int64)
    dst = np.asarray(inputs["dst"]).astype(np.int64)
    dec_src = np.asarray(inputs["dec_src"]).astype(np.int64)
    dec_dst = np.asarray(inputs["dec_dst"]).astype(np.int64)
    W2a = np.asarray(inputs["W2a"], np.float32)
    b2a = np.asarray(inputs["b2a"], np.float32)
    W2b = np.asarray(inputs["W2b"], np.float32)
    b2b = np.asarray(inputs["b2b"], np.float32)
    W3a = np.asarray(inputs["W3a"], np.float32)
    b3a = np.asarray(inputs["b3a"], np.float32)
    W3b = np.asarray(inputs["W3b"], np.float32)
    b3b = np.asarray(inputs["b3b"], np.float32)
    Wp1 = np.asarray(inputs["Wp1"], np.float32)
    bp1 = np.asarray(inputs["bp1"], np.float32)
    Wp2 = np.asarray(inputs["Wp2"], np.float32)
    bp2 = np.asarray(inputs["bp2"], np.float32)

    n = x2.shape[0]
    assert n == 50000 and src.shape == (R, 200000)
    ed = dec_src.shape[0]
    d3 = x3.shape[1]

    # per-edge weights
    ns_arr = np.stack([_deg_norm(src[r], n) for r in range(R)])
    nd_arr = np.stack([_deg_norm(dst[r], n) for r in range(R)])
    src_f = src.ravel()
    dst_f = dst.ravel()
    rel_f = np.repeat(np.arange(R, dtype=np.int64), src.shape[1])
    w_f = (ns_arr[rel_f, src_f] * nd_arr[rel_f, dst_f]).astype(np.float32)
    tile_f = dst_f >> 7
    half_f = (src_f >= HALF).astype(np.int64)

    cnt = np.bincount((tile_f * R + rel_f) * 2 + half_f,
                      minlength=NT * R * 2).reshape(NT, R, 2)
    CH = max(1, -(-int(cnt.max()) // P))
    SLOT = CH * P

    # rank of each edge within its (tile, rel, half) group
    key = (tile_f * R + rel_f) * 2 + half_f
    order = np.lexsort((half_f, rel_f, tile_f))
    ko = key[order]
    starts = np.zeros(NT * R * 16 // 8, np.int64)
    counts_flat = np.bincount(ko, minlength=NT * R * 2)
    starts = np.zeros(NT * R * 2, np.int64)
    np.cumsum(counts_flat[:-1], out=starts[1:])
    rank = np.empty_like(ko)
    rank[:] = np.arange(len(ko)) - starts[ko]
    so, do_, ro, ho, wo = (a[order] for a in (src_f, dst_f, rel_f, half_f, w_f))

    # idx values (relative to half table) and slot assignment
    ival = (so - ho * HALF).astype(np.int64)
    # full idx cube [NT, 2, R, SLOT]; pad pattern applied after filling
    idx_cube = np.zeros((NT, 2, R, SLOT), np.int64)
    idx_cube[tile_f[order], ho, ro, rank] = ival
    valid = np.zeros((NT, 2, R, SLOT), bool)
    valid[tile_f[order], ho, ro, rank] = True

    ia_cube = np.where(valid, idx_cube, 0).astype(np.int16)
    ib_cube = ia_cube

    # host-built one-hot scatter matrices S: [NT, NCHT, 128 slots, 128 dst]
    dloc = np.zeros((NT, 2, R, SLOT), np.int64)
    wval = np.zeros((NT, 2, R, SLOT), np.float32)
    dloc[tile_f[order], ho, ro, rank] = do_ & 127
    wval[tile_f[order], ho, ro, rank] = wo
    SA6 = np.zeros((NT, 2, R, CH, P, P), np.float32)
    np.put_along_axis(
        SA6, dloc.reshape(NT, 2, R, CH, P)[..., None],
        wval.reshape(NT, 2, R, CH, P)[..., None], axis=5)
    smat_all = SA6.reshape(NT, 2 * R * CH, P, P).astype(ml_dtypes.bfloat16)
    del SA6

    # x tables (bf16)
    xcat = np.zeros((NPAD, XC), np.float32)
    xcat[:n, :D2] = x2
    xcat[:n, D2:D2 + d3] = x3
    xcat = xcat.astype(ml_dtypes.bfloat16)
    xA_h = np.ascontiguousarray(xcat[:HALF])
    xB_h = np.ascontiguousarray(xcat[HALF:])

    # layer-A weights [R*NSL, 128, 256]
    WAp = np.zeros((R, NSL, P, 256), np.float32)
    for r in range(R):
        WAp[r, 0] = W2a[r, 0:P]
        WAp[r, 1] = W2a[r, P:2 * P]
        for k in range(2, NSL):
            f0 = k * P - D2
            rows = min(d3 - f0, P) if f0 < d3 else 0
            if rows > 0:
                WAp[r, k, :rows] = W3a[r, f0:f0 + rows]
    WAp = WAp.reshape(R * NSL, P, 256).astype(ml_dtypes.bfloat16)

    # decoder folding
    M = Wp1 @ Wp2
    A2, A3, B2, B3 = M[0:128], M[128:256], M[256:384], M[384:512]
    WMcat = np.zeros((512, P), np.float32)
    for r in range(R):
        WMcat[0:256, r * 16:(r + 1) * 16] = W2b[r] @ np.concatenate([A2, B2], axis=1)
        WMcat[256:512, r * 16:(r + 1) * 16] = W3b[r] @ np.concatenate([A3, B3], axis=1)
    WMp = WMcat.reshape(4, P, P).astype(ml_dtypes.bfloat16)
    c_total = (b2b.sum(0) @ np.concatenate([A2, B2], axis=1)
               + b3b.sum(0) @ np.concatenate([A3, B3], axis=1))
    c_total = c_total[0:8] + c_total[8:16] + bp1 @ Wp2 + bp2
    uvb_rep = np.tile(np.concatenate([np.zeros(8, np.float32),
                                      c_total.astype(np.float32)]), (P, 1))
    bias_rep = np.tile(np.concatenate([b2a.sum(0), b3a.sum(0)]).astype(np.float32), (P, 1))

    # decoder edges
    epc = -(-ed // NC)                       # 50000
    nb = -(-epc // DB)                       # 13
    in_maps = []
    for c in range(NC):
        tsl = slice(c * TPC, (c + 1) * TPC)
        ia_c = _rep128(_wrap16(
            ia_cube[tsl].transpose(0, 1, 2, 3).reshape(TPC, 2, 4, 2 * SLOT)
            .reshape(TPC * 8, 2 * SLOT)))
        ib_c = _rep128(_wrap16(ib_cube[tsl].reshape(TPC * 2, 8 * SLOT)))
        smat_c = np.ascontiguousarray(
            smat_all[tsl].transpose(0, 2, 1, 3).reshape(TPC, P, 2 * R * CH * P))

        e0 = c * epc
        s_pad = np.zeros(nb * DB, np.int64)
        d_pad = np.zeros(nb * DB, np.int64)
        seg = slice(e0, min(e0 + epc, ed))
        ln = seg.stop - seg.start
        s_pad[:ln] = dec_src[seg]
        d_pad[:ln] = dec_dst[seg]
        di = np.concatenate([(s_pad >> 2).reshape(nb, DB),
                             (d_pad >> 2).reshape(nb, DB)], axis=1).astype(np.int16)
        di_c = _rep128(_wrap16(di))          # [128, nb*512]
        mu = (s_pad & 3).astype(np.float32).reshape(nb, 32, P).transpose(0, 2, 1)
        mv = (d_pad & 3).astype(np.float32).reshape(nb, 32, P).transpose(0, 2, 1)
        # [nb, 128, 32] -> repeat 8 -> [nb, 128, 256]
        mu = np.repeat(mu, 8, axis=2)
        mv = np.repeat(mv, 8, axis=2)
        # one-hot select masks per quarter: [nb, 8, 128, 256]
        em = np.stack([(mu == b4).astype(np.float32) for b4 in range(4)]
                      + [(mv == b4).astype(np.float32) for b4 in range(4)], axis=1)
        mk = np.ascontiguousarray(
            em.transpose(2, 0, 1, 3).reshape(P, nb * 8 * 256))
        in_maps.append(dict(
            xA=xA_h, xB=xB_h, WA=WAp, WM=WMp, bias_rep=bias_rep,
            uvb_rep=uvb_rep,
            idx_a=ia_c, idx_b=ib_c, smat=smat_c,
            idx_d=di_c, msk_d=mk,
        ))
    dims = dict(CH=CH, nb=nb, epc=epc, ed=ed)
    return in_maps, dims


_CACHE = {}


def kernel(**inputs):
    in_maps, dims = _prep(inputs)
    key = (dims["CH"], dims["nb"])
    nc = _CACHE.get(key)
    if nc is None:
        nc = _build(dims)
        _CACHE[key] = nc
    res = run_bass_kernel_spmd(nc, in_maps, list(range(NC)))
    epc, ed = dims["epc"], dims["ed"]
    out = np.concatenate(
        [res.results[c]["score_out"][:min(epc, ed - c * epc)] for c in range(NC)], axis=0)
    return np.ascontiguousarray(out.astype(np.float32))


if __name__ == "__main__":
    pass



# revision 28
# speedup vs baseline: 1.6544x; 1.1469x over previous
"""Trainium2 Bass kernel for the 2-layer hetero-GCN + linear edge decoder.

Math restructuring (exact, up to fp reassociation):
  hetero_conv: out = sum_r nd_r*(A_r @ (ns_r*x)) @ W_r + sum_r b_r
    -> per-edge weight w_e = ns_r[src]*nd_r[dst] folded into a one-hot
       scatter matrix S; aggregation z_r^T = g^T @ S runs directly
       transposed on the TensorEngine (no separate transpose step);
       the W_r matmul happens once per dst-tile.
  decoder has NO nonlinearity between Wp1 and Wp2, so
    score[e] = u[src_e] + v[dst_e] + c,  u = feat @ (Wp1@Wp2)[:256],
    v = feat @ (Wp1@Wp2)[256:], folded into the layer-B weight matmul
    (m = h @ WMcat, 16 useful cols per relation).

All matmuls and gathered tables run in bf16 (4x PE throughput vs fp32,
half the gather bytes); PSUM accumulates fp32.  Gathers are merged per
dst-tile: layer A uses 8 gathers/tile (one per relation-pair x half),
layer B 2 gathers/tile, decoder 1 gather/block.  Trailing slots of the
last relation block in each gather are padded with idx=-1 (skipped by
the DMA engine); the true valid count is loaded per-core into a
register via value_load, keeping the program SPMD-uniform.

Sharding: dst-nodes partitioned into 128-node tiles, 49 tiles/core x 8
cores; x tables replicated; AllGather of the per-node message table m
([npad,128] bf16) and of the tiny (u,v) table; decoder edges sharded
evenly.
"""
import os
import sys

for _p in ("/opt/trn_rl_repo", "/root/.axon_site/_ro/trn_rl_repo"):
    if os.path.isdir(_p) and _p not in sys.path:
        sys.path.append(_p)

import numpy as np
import ml_dtypes

import concourse.bass as bass
import concourse.bacc as bacc
import concourse.mybir as mybir
import concourse.tile as tile
from concourse.bass_utils import run_bass_kernel_spmd
from concourse.masks import make_identity

P = 128
NC = 8
R = 8
F32 = mybir.dt.float32
BF16 = mybir.dt.bfloat16
I16 = mybir.dt.int16
I32 = mybir.dt.int32

DYN_COUNT = os.environ.get("K_DYN_COUNT", "0") == "1"

NPAD = 50176          # 392 tiles of 128
TPC = NPAD // (P * NC)  # 49 tiles per core
NT = NPAD // P        # 392
HALF = NPAD // 2      # 25088, split point for int16 gather indices
XC = 640              # padded x row (bf16 -> 1280B, mult of 256B)
NSL = 5               # 640 / 128 k-slices
D2 = 256
DB = 4096             # decoder edges per block


def _deg_norm(idx, n):
    deg = np.bincount(idx, minlength=n).astype(np.float32)
    out = np.zeros(n, np.float32)
    nz = deg > 0
    out[nz] = 1.0 / np.sqrt(np.maximum(deg[nz], 1.0))
    return out


def _wrap16(a):
    # [G, L] -> [16, G*L/16]: idx i of each group -> [i%16, group*L/16 + i//16]
    G, L = a.shape
    return a.reshape(G, L // 16, 16).transpose(2, 0, 1).reshape(16, G * (L // 16))


def _rep128(a16):
    return np.ascontiguousarray(np.tile(a16, (8, 1)))


def _build(dims):
    SL = dims["SL"]             # slots per (rel, half); 256 < SL <= 384
    ORD = 3                     # chunks touched per (rel, half)
    CHH = 8 * SL // P           # chunks per half
    NCHT = 16 * ORD             # S matrices per tile
    nb = dims["nb"]
    tpc = TPC
    ach = 8 * SL // 16          # idx cols per half

    nc = bacc.Bacc("TRN2", target_bir_lowering=False, debug=False)

    xA = nc.declare_dram_parameter("xA", [HALF, XC], BF16, isOutput=False)
    xB = nc.declare_dram_parameter("xB", [NPAD - HALF, XC], BF16, isOutput=False)
    WA = nc.declare_dram_parameter("WA", [R * NSL, P, 256], BF16, isOutput=False)
    WM = nc.declare_dram_parameter("WM", [4, P, P], BF16, isOutput=False)
    bias_rep = nc.declare_dram_parameter("bias_rep", [P, 512], F32, isOutput=False)
    uvb_rep = nc.declare_dram_parameter("uvb_rep", [P, 16], F32, isOutput=False)
    idx_a = nc.declare_dram_parameter("idx_a", [P, tpc * 2 * ach], I16, isOutput=False)
    smat = nc.declare_dram_parameter(
        "smat", [tpc, P, NCHT * P], BF16, isOutput=False)
    idx_d = nc.declare_dram_parameter("idx_d", [P, nb * 512], I16, isOutput=False)
    msk_d = nc.declare_dram_parameter("msk_d", [P, nb * 8 * 256], F32, isOutput=False)
    score_out = nc.declare_dram_parameter("score_out", [nb * DB, 8], F32, isOutput=True)

    AL = mybir.AluOpType

    with tile.TileContext(nc) as tc:
        with (
            tc.tile_pool(name="cpool", bufs=1) as cp,
            tc.tile_pool(name="dram", bufs=1, space="DRAM") as dp,
        ):
            m_shard = dp.tile([tpc * P, P], BF16)
            m_full = dp.tile([NPAD, P], BF16, addr_space="Shared")
            uv_shard = dp.tile([tpc * P, 16], F32)
            uv_full = dp.tile([NPAD // 4, 64], F32, addr_space="Shared")

            ident = cp.tile([P, P], BF16)
            make_identity(nc, ident[:])
            bias_t = cp.tile([P, 512], F32)
            nc.sync.dma_start(out=bias_t[:], in_=bias_rep[:, :])
            uvb_t = cp.tile([P, 16], F32)
            nc.sync.dma_start(out=uvb_t[:], in_=uvb_rep[:, :])
            wa_t = []
            for i in range(R * NSL):
                w_i = cp.tile([P, 256], BF16, tag=f"wa{i}")
                nc.sync.dma_start(out=w_i[:], in_=WA[i, :, :])
                wa_t.append(w_i)
            wm_t = []
            for k in range(4):
                w_k = cp.tile([P, P], BF16, tag=f"wm{k}")
                nc.sync.dma_start(out=w_k[:], in_=WM[k, :, :])
                wm_t.append(w_k)

            # ---------------- layer A ----------------
            GBUFS = 3
            with (
                tc.tile_pool(name="mpool", bufs=2) as mp,
                tc.tile_pool(name="gpool", bufs=GBUFS) as ga,
                tc.tile_pool(name="spool", bufs=2) as sp,
                tc.tile_pool(name="zpool", bufs=3) as zp,
                tc.tile_pool(name="pszpool", bufs=2, space="PSUM") as psz,
                tc.tile_pool(name="pshpool", bufs=1, space="PSUM") as psh,
                tc.tile_pool(name="pstpool", bufs=2, space="PSUM") as pst,
                tc.tile_pool(name="psmpool", bufs=1, space="PSUM") as psm,
            ):
                for t in range(tpc):
                    ia_t = mp.tile([P, 2 * ach], I16, tag="ia")
                    nc.sync.dma_start(out=ia_t[:], in_=idx_a[:, t * 2 * ach:(t + 1) * 2 * ach])
                    s_all = sp.tile([P, NCHT * P], BF16, tag="sall")
                    nc.sync.dma_start(out=s_all[:], in_=smat[t, :, :])
                    gt = []
                    for h in (0, 1):   # one merged gather per half
                        g = ga.tile([P, CHH, XC], BF16, tag="g")
                        nc.gpsimd.dma_gather(
                            out_ap=g[:], in_ap=(xA if h == 0 else xB)[:, :],
                            idxs_ap=ia_t[:, h * ach:(h + 1) * ach],
                            num_idxs=8 * SL, num_idxs_reg=8 * SL,
                            elem_size=XC, single_packet=False)
                        gt.append(g)
                    h2 = psh.tile([P, 256], F32, tag="h2")
                    h3 = psh.tile([P, 256], F32, tag="h3")
                    for r in range(R):
                        seq = []
                        for h in (0, 1):
                            for o in range(ORD):
                                lc = (SL * r) // P + o
                                seq.append((gt[h], lc, (h * 8 + r) * ORD + o))
                        for k in range(NSL):
                            ztk = psz.tile([P, P], F32, tag="ztk")
                            for i, (gtile, lc, gch) in enumerate(seq):
                                nc.tensor.matmul(
                                    ztk[:], gtile[:, lc, k * P:(k + 1) * P],
                                    s_all[:, gch * P:(gch + 1) * P],
                                    start=(i == 0), stop=(i == 2 * ORD - 1))
                            zsk = zp.tile([P, P], BF16, tag="zsk")
                            nc.scalar.copy(out=zsk[:], in_=ztk[:])
                            nc.tensor.matmul(
                                (h2 if k < 2 else h3)[:],
                                zsk[:], wa_t[r * NSL + k][:],
                                start=(r == 0 and k in (0, 2)),
                                stop=(r == R - 1 and k in (1, NSL - 1)))
                    hsb = zp.tile([P, 512], BF16, tag="hsb")
                    nc.vector.tensor_tensor(out=hsb[:, 0:256], in0=h2[:],
                                            in1=bias_t[:, 0:256], op=AL.add)
                    nc.vector.tensor_tensor(out=hsb[:, 256:512], in0=h3[:],
                                            in1=bias_t[:, 256:512], op=AL.add)
                    nc.vector.tensor_scalar_max(out=hsb[:], in0=hsb[:], scalar1=0.0)
                    ht4 = zp.tile([P, 4, P], BF16, tag="ht4")
                    for k in range(4):
                        tp = pst.tile([P, P], BF16, tag="tp")
                        nc.tensor.transpose(out=tp[:], in_=hsb[:, k * P:(k + 1) * P],
                                            identity=ident[:])
                        nc.scalar.copy(out=ht4[:, k, :], in_=tp[:])
                    mps = psm.tile([P, P], F32, tag="mps")
                    for k in range(4):
                        nc.tensor.matmul(mps[:], ht4[:, k, :], wm_t[k][:],
                                         start=(k == 0), stop=(k == 3))
                    msb = zp.tile([P, P], BF16, tag="msb")
                    nc.vector.tensor_copy(out=msb[:], in_=mps[:])
                    nc.sync.dma_start(out=m_shard[t * P:(t + 1) * P, :], in_=msb[:])

            nc.gpsimd.collective_compute(
                "AllGather", mybir.AluOpType.bypass,
                replica_groups=[list(range(NC))],
                ins=[m_shard[:, :]], outs=[m_full[:, :]])

            # ---------------- layer B ----------------
            GBBUFS = 4
            with (
                tc.tile_pool(name="mpoolb", bufs=2) as mp,
                tc.tile_pool(name="gpoolb", bufs=GBBUFS) as gb_p,
                tc.tile_pool(name="spoolb", bufs=2) as sp,
                tc.tile_pool(name="zpoolb", bufs=2) as zp,
                tc.tile_pool(name="psupool", bufs=2, space="PSUM") as psu,
            ):
                for t in range(tpc):
                    ib_t = mp.tile([P, 2 * ach], I16, tag="ib")
                    nc.sync.dma_start(out=ib_t[:], in_=idx_a[:, t * 2 * ach:(t + 1) * 2 * ach])
                    s_all = sp.tile([P, NCHT * P], BF16, tag="sallb")
                    nc.sync.dma_start(out=s_all[:], in_=smat[t, :, :])
                    gbt = []
                    for h in (0, 1):
                        g = gb_p.tile([P, CHH, P], BF16, tag="gb")
                        nc.gpsimd.dma_gather(
                            out_ap=g[:],
                            in_ap=(m_full[0:HALF, :] if h == 0 else m_full[HALF:NPAD, :]),
                            idxs_ap=ib_t[:, h * ach:(h + 1) * ach],
                            num_idxs=8 * SL, num_idxs_reg=8 * SL, elem_size=P,
                            single_packet=False)
                        gbt.append(g)
                    uvp = psu.tile([P, 16], F32, tag="uvp")
                    for h in (0, 1):
                        for r in range(R):
                            for o in range(ORD):
                                gch = (h * 8 + r) * ORD + o
                                nc.tensor.matmul(
                                    uvp[:], s_all[:, gch * P:(gch + 1) * P],
                                    gbt[h][:, (SL * r) // P + o, r * 16:(r + 1) * 16],
                                    start=(gch == 0), stop=(gch == NCHT - 1))
                    uvsb = zp.tile([P, 16], F32, tag="uvsb")
                    nc.vector.tensor_tensor(out=uvsb[:], in0=uvp[:], in1=uvb_t[:],
                                            op=AL.add)
                    nc.sync.dma_start(out=uv_shard[t * P:(t + 1) * P, :], in_=uvsb[:])

            nc.gpsimd.collective_compute(
                "AllGather", mybir.AluOpType.bypass,
                replica_groups=[list(range(NC))],
                ins=[uv_shard[:, :]], outs=[uv_full[:, :]])

            # ---------------- decoder ----------------
            with (
                tc.tile_pool(name="dmp", bufs=2) as mp,
                tc.tile_pool(name="dgp", bufs=2) as gp,
                tc.tile_pool(name="dvp", bufs=2) as vp,
            ):
                sview = score_out.ap().rearrange("(B j p) d -> B p j d", p=P, j=32)
                for b in range(nb):
                    id_t = mp.tile([P, 512], I16, tag="id")
                    nc.sync.dma_start(out=id_t[:], in_=idx_d[:, b * 512:(b + 1) * 512])
                    mk_t = mp.tile([P, 8, 32, 8], F32, tag="mk")
                    nc.sync.dma_start(
                        out=mk_t[:],
                        in_=msk_d[:, b * 2048:(b + 1) * 2048])
                    gd = gp.tile([P, 64, 64], F32, tag="gd")
                    nc.gpsimd.dma_gather(
                        out_ap=gd[:, 0:32, :], in_ap=uv_full[:, :],
                        idxs_ap=id_t[:, 0:256],
                        num_idxs=4096, num_idxs_reg=4096, elem_size=64,
                        single_packet=False)
                    nc.gpsimd.dma_gather(
                        out_ap=gd[:, 32:64, :], in_ap=uv_full[:, :],
                        idxs_ap=id_t[:, 256:512],
                        num_idxs=4096, num_idxs_reg=4096, elem_size=64,
                        single_packet=False)
                    acc = vp.tile([P, 32, 8], F32, tag="acc")
                    accv = vp.tile([P, 32, 8], F32, tag="accv")
                    for b4 in range(4):
                        if b4 == 0:
                            nc.vector.tensor_tensor(
                                out=acc[:], in0=mk_t[:, 0, :, :],
                                in1=gd[:, 0:32, 0:8], op=AL.mult)
                        else:
                            eq = vp.tile([P, 32, 8], F32, tag="eq")
                            nc.vector.tensor_tensor(
                                out=eq[:], in0=mk_t[:, b4, :, :],
                                in1=gd[:, 0:32, b4 * 16:b4 * 16 + 8], op=AL.mult)
                            nc.vector.tensor_tensor(out=acc[:], in0=acc[:], in1=eq[:],
                                                    op=AL.add)
                    for b4 in range(4):
                        if b4 == 0:
                            nc.vector.tensor_tensor(
                                out=accv[:], in0=mk_t[:, 4, :, :],
                                in1=gd[:, 32:64, 8:16], op=AL.mult)
                        else:
                            eq = vp.tile([P, 32, 8], F32, tag="eq")
                            nc.vector.tensor_tensor(
                                out=eq[:], in0=mk_t[:, 4 + b4, :, :],
                                in1=gd[:, 32:64, b4 * 16 + 8:b4 * 16 + 16], op=AL.mult)
                            nc.vector.tensor_tensor(out=accv[:], in0=accv[:], in1=eq[:],
                                                    op=AL.add)
                    nc.vector.tensor_tensor(out=acc[:], in0=acc[:], in1=accv[:],
                                            op=AL.add)
                    nc.sync.dma_start(out=sview[b], in_=acc[:])
    nc.finalize()
    return nc


def _prep(inputs):
    x2 = np.asarray(inputs["node2_features"], np.float32)
    x3 = np.asarray(inputs["mpnn_features"], np.float32)
    src = np.asarray(inputs["src"]).astype(np.int64)
    dst = np.asarray(inputs["dst"]).astype(np.int64)
    dec_src = np.asarray(inputs["dec_src"]).astype(np.int64)
    dec_dst = np.asarray(inputs["dec_dst"]).astype(np.int64)
    W2a = np.asarray(inputs["W2a"], np.float32)
    b2a = np.asarray(inputs["b2a"], np.float32)
    W2b = np.asarray(inputs["W2b"], np.float32)
    b2b = np.asarray(inputs["b2b"], np.float32)
    W3a = np.asarray(inputs["W3a"], np.float32)
    b3a = np.asarray(inputs["b3a"], np.float32)
    W3b = np.asarray(inputs["W3b"], np.float32)
    b3b = np.asarray(inputs["b3b"], np.float32)
    Wp1 = np.asarray(inputs["Wp1"], np.float32)
    bp1 = np.asarray(inputs["bp1"], np.float32)
    Wp2 = np.asarray(inputs["Wp2"], np.float32)
    bp2 = np.asarray(inputs["bp2"], np.float32)

    n = x2.shape[0]
    assert n == 50000 and src.shape == (R, 200000)
    ed = dec_src.shape[0]
    d3 = x3.shape[1]

    # per-edge weights
    ns_arr = np.stack([_deg_norm(src[r], n) for r in range(R)])
    nd_arr = np.stack([_deg_norm(dst[r], n) for r in range(R)])
    src_f = src.ravel()
    dst_f = dst.ravel()
    rel_f = np.repeat(np.arange(R, dtype=np.int64), src.shape[1])
    w_f = (ns_arr[rel_f, src_f] * nd_arr[rel_f, dst_f]).astype(np.float32)
    tile_f = dst_f >> 7
    half_f = (src_f >= HALF).astype(np.int64)

    cnt = np.bincount((tile_f * R + rel_f) * 2 + half_f,
                      minlength=NT * R * 2).reshape(NT, R, 2)
    # slots per (rel, half): tight multiple of 64; ORD=3 chunk-straddle
    # structure requires 256 < SL <= 384.
    SL = max(320, -(-int(cnt.max()) // 64) * 64)
    assert SL <= 384, f"group max {cnt.max()} exceeds SL ceiling"
    ORD = 3

    # rank of each edge within its (tile, rel, half) group
    key = (tile_f * R + rel_f) * 2 + half_f
    order = np.lexsort((half_f, rel_f, tile_f))
    ko = key[order]
    counts_flat = np.bincount(ko, minlength=NT * R * 2)
    starts = np.zeros(NT * R * 2, np.int64)
    np.cumsum(counts_flat[:-1], out=starts[1:])
    rank = np.empty_like(ko)
    rank[:] = np.arange(len(ko)) - starts[ko]
    so, do_, ro, ho, wo = (a[order] for a in (src_f, dst_f, rel_f, half_f, w_f))

    # idx values (relative to half table) and slot assignment
    ival = (so - ho * HALF).astype(np.int64)
    ia_cube = np.zeros((NT, 2, R * SL), np.int16)
    ia_cube[tile_f[order], ho, ro * SL + rank] = ival.astype(np.int16)

    # host-built one-hot scatter matrices S, per (half, rel, straddle-ord):
    # chunk c covers half-slots [128c, 128c+128); rel r occupies
    # [SL*r, SL*r+SL) -> ords 0..2 relative to c0 = (SL*r)//128.
    slot_ih = SL * ro + rank
    chunk_e = slot_ih >> 7
    row_e = slot_ih & 127
    ord_e = chunk_e - (SL * ro) // P
    S6 = np.zeros((NT, 2, R, ORD, P, P), np.float32)
    S6[tile_f[order], ho, ro, ord_e, row_e, do_ & 127] = wo
    smat_all = S6.reshape(NT, 2 * R * ORD, P, P).astype(ml_dtypes.bfloat16)
    del S6

    # x tables (bf16)
    xcat = np.zeros((NPAD, XC), np.float32)
    xcat[:n, :D2] = x2
    xcat[:n, D2:D2 + d3] = x3
    xcat = xcat.astype(ml_dtypes.bfloat16)
    xA_h = np.ascontiguousarray(xcat[:HALF])
    xB_h = np.ascontiguousarray(xcat[HALF:])

    # layer-A weights [R*NSL, 128, 256]
    WAp = np.zeros((R, NSL, P, 256), np.float32)
    for r in range(R):
        WAp[r, 0] = W2a[r, 0:P]
        WAp[r, 1] = W2a[r, P:2 * P]
        for k in range(2, NSL):
            f0 = k * P - D2
            rows = min(d3 - f0, P) if f0 < d3 else 0
            if rows > 0:
                WAp[r, k, :rows] = W3a[r, f0:f0 + rows]
    WAp = WAp.reshape(R * NSL, P, 256).astype(ml_dtypes.bfloat16)

    # decoder folding
    M = Wp1 @ Wp2
    A2, A3, B2, B3 = M[0:128], M[128:256], M[256:384], M[384:512]
    WMcat = np.zeros((512, P), np.float32)
    for r in range(R):
        WMcat[0:256, r * 16:(r + 1) * 16] = W2b[r] @ np.concatenate([A2, B2], axis=1)
        WMcat[256:512, r * 16:(r + 1) * 16] = W3b[r] @ np.concatenate([A3, B3], axis=1)
    WMp = WMcat.reshape(4, P, P).astype(ml_dtypes.bfloat16)
    c_total = (b2b.sum(0) @ np.concatenate([A2, B2], axis=1)
               + b3b.sum(0) @ np.concatenate([A3, B3], axis=1))
    c_total = c_total[0:8] + c_total[8:16] + bp1 @ Wp2 + bp2
    uvb_rep = np.tile(np.concatenate([np.zeros(8, np.float32),
                                      c_total.astype(np.float32)]), (P, 1))
    bias_rep = np.tile(np.concatenate([b2a.sum(0), b3a.sum(0)]).astype(np.float32), (P, 1))

    # decoder edges
    epc = -(-ed // NC)                       # 50000
    nb = -(-epc // DB)                       # 13
    in_maps = []
    for c in range(NC):
        tsl = slice(c * TPC, (c + 1) * TPC)
        ia_c = _rep128(_wrap16(ia_cube[tsl].reshape(TPC * 2, 8 * SL)))
        smat_c = np.ascontiguousarray(
            smat_all[tsl].transpose(0, 2, 1, 3).reshape(TPC, P, 2 * R * ORD * P))

        e0 = c * epc
        s_pad = np.zeros(nb * DB, np.int64)
        d_pad = np.zeros(nb * DB, np.int64)
        seg = slice(e0, min(e0 + epc, ed))
        ln = seg.stop - seg.start
        s_pad[:ln] = dec_src[seg]
        d_pad[:ln] = dec_dst[seg]
        di = np.concatenate([(s_pad >> 2).reshape(nb, DB),
                             (d_pad >> 2).reshape(nb, DB)], axis=1).astype(np.int16)
        di_c = _rep128(_wrap16(di))          # [128, nb*512]
        mu = (s_pad & 3).astype(np.float32).reshape(nb, 32, P).transpose(0, 2, 1)
        mv = (d_pad & 3).astype(np.float32).reshape(nb, 32, P).transpose(0, 2, 1)
        # [nb, 128, 32] -> repeat 8 -> [nb, 128, 256]
        mu = np.repeat(mu, 8, axis=2)
        mv = np.repeat(mv, 8, axis=2)
        # one-hot select masks per quarter: [nb, 8, 128, 256]
        em = np.stack([(mu == b4).astype(np.float32) for b4 in range(4)]
                      + [(mv == b4).astype(np.float32) for b4 in range(4)], axis=1)
        mk = np.ascontiguousarray(
            em.transpose(2, 0, 1, 3).reshape(P, nb * 8 * 256))
        in_maps.append(dict(
            xA=xA_h, xB=xB_h, WA=WAp, WM=WMp, bias_rep=bias_rep,
            uvb_rep=uvb_rep,
            idx_a=ia_c, smat=smat_c,
            idx_d=di_c, msk_d=mk,
        ))
    dims = dict(SL=SL, nb=nb, epc=epc, ed=ed)
    return in_maps, dims


_CACHE = {}


def kernel(**inputs):
    in_maps, dims = _prep(inputs)
    key = (dims["SL"], dims["nb"])
    nc = _CACHE.get(key)
    if nc is None:
        nc = _build(dims)
        _CACHE[key] = nc
    res = run_bass_kernel_spmd(nc, in_maps, list(range(NC)))
    epc, ed = dims["epc"], dims["ed"]
    out = np.concatenate(
        [res.results[c]["score_out"][:min(epc, ed - c * epc)] for c in range(NC)], axis=0)
    return np.ascontiguousarray(out.astype(np.float32))


if __name__ == "__main__":
    pass



# revision 29
# speedup vs baseline: 1.9253x; 1.1637x over previous
"""Trainium2 Bass kernel for the 2-layer hetero-GCN + linear edge decoder.

Math restructuring (exact, up to fp reassociation):
  hetero_conv: out = sum_r nd_r*(A_r @ (ns_r*x)) @ W_r + sum_r b_r
    -> per-edge weight w_e = ns_r[src]*nd_r[dst] folded into a one-hot
       scatter matrix S; aggregation z_r^T = g^T @ S runs directly
       transposed on the TensorEngine (no separate transpose step);
       the W_r matmul happens once per dst-tile.
  decoder has NO nonlinearity between Wp1 and Wp2, so
    score[e] = u[src_e] + v[dst_e] + c,  u = feat @ (Wp1@Wp2)[:256],
    v = feat @ (Wp1@Wp2)[256:], folded into the layer-B weight matmul
    (m = h @ WMcat, 16 useful cols per relation).

All matmuls and gathered tables run in bf16 (4x PE throughput vs fp32,
half the gather bytes); PSUM accumulates fp32.  Gathers are merged per
dst-tile: layer A uses 8 gathers/tile (one per relation-pair x half),
layer B 2 gathers/tile, decoder 1 gather/block.  Trailing slots of the
last relation block in each gather are padded with idx=-1 (skipped by
the DMA engine); the true valid count is loaded per-core into a
register via value_load, keeping the program SPMD-uniform.

Sharding: dst-nodes partitioned into 128-node tiles, 49 tiles/core x 8
cores; x tables replicated; AllGather of the per-node message table m
([npad,128] bf16) and of the tiny (u,v) table; decoder edges sharded
evenly.
"""
import os
import sys

for _p in ("/opt/trn_rl_repo", "/root/.axon_site/_ro/trn_rl_repo"):
    if os.path.isdir(_p) and _p not in sys.path:
        sys.path.append(_p)

import numpy as np
import ml_dtypes

import concourse.bass as bass
import concourse.bacc as bacc
import concourse.mybir as mybir
import concourse.tile as tile
from concourse.bass_utils import run_bass_kernel_spmd
from concourse.masks import make_identity

P = 128
NC = 8
R = 8
F32 = mybir.dt.float32
BF16 = mybir.dt.bfloat16
I16 = mybir.dt.int16
I32 = mybir.dt.int32

DYN_COUNT = os.environ.get("K_DYN_COUNT", "0") == "1"

NPAD = 50176          # 392 tiles of 128
TPC = NPAD // (P * NC)  # 49 tiles per core
NT = NPAD // P        # 392
HALF = NPAD // 2      # 25088, split point for int16 gather indices
XC = 640              # padded x row (bf16 -> 1280B, mult of 256B)
NSL = 5               # 640 / 128 k-slices
D2 = 256
DB = 4096             # decoder edges per block


def _deg_norm(idx, n):
    deg = np.bincount(idx, minlength=n).astype(np.float32)
    out = np.zeros(n, np.float32)
    nz = deg > 0
    out[nz] = 1.0 / np.sqrt(np.maximum(deg[nz], 1.0))
    return out


def _wrap16(a):
    # [G, L] -> [16, G*L/16]: idx i of each group -> [i%16, group*L/16 + i//16]
    G, L = a.shape
    return a.reshape(G, L // 16, 16).transpose(2, 0, 1).reshape(16, G * (L // 16))


def _rep128(a16):
    return np.ascontiguousarray(np.tile(a16, (8, 1)))


def _build(dims):
    SL = dims["SL"]             # slots per (rel, half); 256 < SL <= 384
    ORD = 3                     # chunks touched per (rel, half)
    CHH = 8 * SL // P           # chunks per half
    NCHT = 16 * ORD             # S matrices per tile
    nb = dims["nb"]
    tpc = TPC
    ach = 8 * SL // 16          # idx cols per half

    nc = bacc.Bacc("TRN2", target_bir_lowering=False, debug=False)

    xA = nc.declare_dram_parameter("xA", [HALF, XC], BF16, isOutput=False)
    xB = nc.declare_dram_parameter("xB", [NPAD - HALF, XC], BF16, isOutput=False)
    WA = nc.declare_dram_parameter("WA", [R * NSL, P, 256], BF16, isOutput=False)
    WM = nc.declare_dram_parameter("WM", [4, P, P], BF16, isOutput=False)
    bias_rep = nc.declare_dram_parameter("bias_rep", [P, 512], F32, isOutput=False)
    uvb_rep = nc.declare_dram_parameter("uvb_rep", [P, 16], F32, isOutput=False)
    idx_a = nc.declare_dram_parameter("idx_a", [P, tpc * 2 * ach], I16, isOutput=False)
    smat = nc.declare_dram_parameter(
        "smat", [tpc, P, NCHT * P], BF16, isOutput=False)
    idx_d = nc.declare_dram_parameter("idx_d", [P, nb * 512], I16, isOutput=False)
    msk_d = nc.declare_dram_parameter("msk_d", [P, nb * 8 * 256], F32, isOutput=False)
    score_out = nc.declare_dram_parameter("score_out", [nb * DB, 8], F32, isOutput=True)

    AL = mybir.AluOpType

    with tile.TileContext(nc) as tc:
        with (
            tc.tile_pool(name="cpool", bufs=1) as cp,
            tc.tile_pool(name="dram", bufs=1, space="DRAM") as dp,
        ):
            m_shard = dp.tile([tpc * P, P], BF16)
            m_full = dp.tile([NPAD, P], BF16, addr_space="Shared")
            uv_shard = dp.tile([tpc * P, 16], F32)
            uv_full = dp.tile([NPAD // 4, 64], F32, addr_space="Shared")

            ident = cp.tile([P, P], BF16)
            make_identity(nc, ident[:])
            bias_t = cp.tile([P, 512], F32)
            nc.sync.dma_start(out=bias_t[:], in_=bias_rep[:, :])
            uvb_t = cp.tile([P, 16], F32)
            nc.sync.dma_start(out=uvb_t[:], in_=uvb_rep[:, :])
            wa_t = []
            for i in range(R * NSL):
                w_i = cp.tile([P, 256], BF16, tag=f"wa{i}")
                nc.sync.dma_start(out=w_i[:], in_=WA[i, :, :])
                wa_t.append(w_i)
            wm_t = []
            for k in range(4):
                w_k = cp.tile([P, P], BF16, tag=f"wm{k}")
                nc.sync.dma_start(out=w_k[:], in_=WM[k, :, :])
                wm_t.append(w_k)

            # ---------------- layer A ----------------
            GBUFS = 5
            with (
                tc.tile_pool(name="mpool", bufs=2) as mp,
                tc.tile_pool(name="gpool", bufs=GBUFS) as ga,
                tc.tile_pool(name="spool", bufs=2) as sp,
                tc.tile_pool(name="zpool", bufs=3) as zp,
                tc.tile_pool(name="pszpool", bufs=2, space="PSUM") as psz,
                tc.tile_pool(name="pshpool", bufs=1, space="PSUM") as psh,
                tc.tile_pool(name="pstpool", bufs=2, space="PSUM") as pst,
                tc.tile_pool(name="psmpool", bufs=1, space="PSUM") as psm,
            ):
                for t in range(tpc):
                    ia_t = mp.tile([P, 2 * ach], I16, tag="ia")
                    nc.sync.dma_start(out=ia_t[:], in_=idx_a[:, t * 2 * ach:(t + 1) * 2 * ach])
                    s_all = sp.tile([P, NCHT * P], BF16, tag="sall")
                    nc.sync.dma_start(out=s_all[:], in_=smat[t, :, :])
                    gt = []
                    for h in (0, 1):   # one merged gather per half
                        g = ga.tile([P, CHH, XC], BF16, tag="g")
                        nc.gpsimd.dma_gather(
                            out_ap=g[:], in_ap=(xA if h == 0 else xB)[:, :],
                            idxs_ap=ia_t[:, h * ach:(h + 1) * ach],
                            num_idxs=8 * SL, num_idxs_reg=8 * SL,
                            elem_size=XC, single_packet=False)
                        gt.append(g)
                    h2 = psh.tile([P, 256], F32, tag="h2")
                    h3 = psh.tile([P, 256], F32, tag="h3")
                    for r in range(R):
                        seq = []
                        for h in (0, 1):
                            for o in range(ORD):
                                lc = (SL * r) // P + o
                                seq.append((gt[h], lc, (h * 8 + r) * ORD + o))
                        for k in range(NSL):
                            ztk = psz.tile([P, P], F32, tag="ztk")
                            for i, (gtile, lc, gch) in enumerate(seq):
                                nc.tensor.matmul(
                                    ztk[:], gtile[:, lc, k * P:(k + 1) * P],
                                    s_all[:, gch * P:(gch + 1) * P],
                                    start=(i == 0), stop=(i == 2 * ORD - 1))
                            zsk = zp.tile([P, P], BF16, tag="zsk")
                            nc.scalar.copy(out=zsk[:], in_=ztk[:])
                            nc.tensor.matmul(
                                (h2 if k < 2 else h3)[:],
                                zsk[:], wa_t[r * NSL + k][:],
                                start=(r == 0 and k in (0, 2)),
                                stop=(r == R - 1 and k in (1, NSL - 1)))
                    hsb = zp.tile([P, 512], BF16, tag="hsb")
                    nc.vector.tensor_tensor(out=hsb[:, 0:256], in0=h2[:],
                                            in1=bias_t[:, 0:256], op=AL.add)
                    nc.vector.tensor_tensor(out=hsb[:, 256:512], in0=h3[:],
                                            in1=bias_t[:, 256:512], op=AL.add)
                    nc.vector.tensor_scalar_max(out=hsb[:], in0=hsb[:], scalar1=0.0)
                    ht4 = zp.tile([P, 4, P], BF16, tag="ht4")
                    for k in range(4):
                        tp = pst.tile([P, P], BF16, tag="tp")
                        nc.tensor.transpose(out=tp[:], in_=hsb[:, k * P:(k + 1) * P],
                                            identity=ident[:])
                        nc.scalar.copy(out=ht4[:, k, :], in_=tp[:])
                    mps = psm.tile([P, P], F32, tag="mps")
                    for k in range(4):
                        nc.tensor.matmul(mps[:], ht4[:, k, :], wm_t[k][:],
                                         start=(k == 0), stop=(k == 3))
                    msb = zp.tile([P, P], BF16, tag="msb")
                    nc.vector.tensor_copy(out=msb[:], in_=mps[:])
                    nc.sync.dma_start(out=m_shard[t * P:(t + 1) * P, :], in_=msb[:])

            nc.gpsimd.collective_compute(
                "AllGather", mybir.AluOpType.bypass,
                replica_groups=[list(range(NC))],
                ins=[m_shard[:, :]], outs=[m_full[:, :]])

            # ---------------- layer B ----------------
            GBBUFS = 4
            with (
                tc.tile_pool(name="mpoolb", bufs=2) as mp,
                tc.tile_pool(name="gpoolb", bufs=GBBUFS) as gb_p,
                tc.tile_pool(name="spoolb", bufs=2) as sp,
                tc.tile_pool(name="zpoolb", bufs=2) as zp,
                tc.tile_pool(name="psupool", bufs=2, space="PSUM") as psu,
            ):
                for t in range(tpc):
                    ib_t = mp.tile([P, 2 * ach], I16, tag="ib")
                    nc.sync.dma_start(out=ib_t[:], in_=idx_a[:, t * 2 * ach:(t + 1) * 2 * ach])
                    s_all = sp.tile([P, NCHT * P], BF16, tag="sallb")
                    nc.sync.dma_start(out=s_all[:], in_=smat[t, :, :])
                    gbt = []
                    for h in (0, 1):
                        g = gb_p.tile([P, CHH, P], BF16, tag="gb")
                        nc.gpsimd.dma_gather(
                            out_ap=g[:],
                            in_ap=(m_full[0:HALF, :] if h == 0 else m_full[HALF:NPAD, :]),
                            idxs_ap=ib_t[:, h * ach:(h + 1) * ach],
                            num_idxs=8 * SL, num_idxs_reg=8 * SL, elem_size=P,
                            single_packet=False)
                        gbt.append(g)
                    uvp = psu.tile([P, 16], F32, tag="uvp")
                    for h in (0, 1):
                        for r in range(R):
                            for o in range(ORD):
                                gch = (h * 8 + r) * ORD + o
                                nc.tensor.matmul(
                                    uvp[:], s_all[:, gch * P:(gch + 1) * P],
                                    gbt[h][:, (SL * r) // P + o, r * 16:(r + 1) * 16],
                                    start=(gch == 0), stop=(gch == NCHT - 1))
                    uvsb = zp.tile([P, 16], F32, tag="uvsb")
                    nc.vector.tensor_tensor(out=uvsb[:], in0=uvp[:], in1=uvb_t[:],
                                            op=AL.add)
                    nc.sync.dma_start(out=uv_shard[t * P:(t + 1) * P, :], in_=uvsb[:])

            nc.gpsimd.collective_compute(
                "AllGather", mybir.AluOpType.bypass,
                replica_groups=[list(range(NC))],
                ins=[uv_shard[:, :]], outs=[uv_full[:, :]])

            # ---------------- decoder ----------------
            with (
                tc.tile_pool(name="dmp", bufs=2) as mp,
                tc.tile_pool(name="dgp", bufs=2) as gp,
                tc.tile_pool(name="dvp", bufs=2) as vp,
            ):
                sview = score_out.ap().rearrange("(B j p) d -> B p j d", p=P, j=32)
                for b in range(nb):
                    id_t = mp.tile([P, 512], I16, tag="id")
                    nc.sync.dma_start(out=id_t[:], in_=idx_d[:, b * 512:(b + 1) * 512])
                    mk_t = mp.tile([P, 8, 32, 8], F32, tag="mk")
                    nc.sync.dma_start(
                        out=mk_t[:],
                        in_=msk_d[:, b * 2048:(b + 1) * 2048])
                    gd = gp.tile([P, 64, 64], F32, tag="gd")
                    nc.gpsimd.dma_gather(
                        out_ap=gd[:, 0:32, :], in_ap=uv_full[:, :],
                        idxs_ap=id_t[:, 0:256],
                        num_idxs=4096, num_idxs_reg=4096, elem_size=64,
                        single_packet=False)
                    nc.gpsimd.dma_gather(
                        out_ap=gd[:, 32:64, :], in_ap=uv_full[:, :],
                        idxs_ap=id_t[:, 256:512],
                        num_idxs=4096, num_idxs_reg=4096, elem_size=64,
                        single_packet=False)
                    acc = vp.tile([P, 32, 8], F32, tag="acc")
                    accv = vp.tile([P, 32, 8], F32, tag="accv")
                    for b4 in range(4):
                        if b4 == 0:
                            nc.vector.tensor_tensor(
                                out=acc[:], in0=mk_t[:, 0, :, :],
                                in1=gd[:, 0:32, 0:8], op=AL.mult)
                        else:
                            eq = vp.tile([P, 32, 8], F32, tag="eq")
                            nc.vector.tensor_tensor(
                                out=eq[:], in0=mk_t[:, b4, :, :],
                                in1=gd[:, 0:32, b4 * 16:b4 * 16 + 8], op=AL.mult)
                            nc.vector.tensor_tensor(out=acc[:], in0=acc[:], in1=eq[:],
                                                    op=AL.add)
                    for b4 in range(4):
                        if b4 == 0:
                            nc.vector.tensor_tensor(
                                out=accv[:], in0=mk_t[:, 4, :, :],
                                in1=gd[:, 32:64, 8:16], op=AL.mult)
                        else:
                            eq = vp.tile([P, 32, 8], F32, tag="eq")
                            nc.vector.tensor_tensor(
                                out=eq[:], in0=mk_t[:, 4 + b4, :, :],
                                in1=gd[:, 32:64, b4 * 16 + 8:b4 * 16 + 16], op=AL.mult)
                            nc.vector.tensor_tensor(out=accv[:], in0=accv[:], in1=eq[:],
                                                    op=AL.add)
                    nc.vector.tensor_tensor(out=acc[:], in0=acc[:], in1=accv[:],
                                            op=AL.add)
                    nc.sync.dma_start(out=sview[b], in_=acc[:])
    nc.finalize()
    return nc


def _prep(inputs):
    x2 = np.asarray(inputs["node2_features"], np.float32)
    x3 = np.asarray(inputs["mpnn_features"], np.float32)
    src = np.asarray(inputs["src"]).astype(np.int64)
    dst = np.asarray(inputs["dst"]).astype(np.int64)
    dec_src = np.asarray(inputs["dec_src"]).astype(np.int64)
    dec_dst = np.asarray(inputs["dec_dst"]).astype(np.int64)
    W2a = np.asarray(inputs["W2a"], np.float32)
    b2a = np.asarray(inputs["b2a"], np.float32)
    W2b = np.asarray(inputs["W2b"], np.float32)
    b2b = np.asarray(inputs["b2b"], np.float32)
    W3a = np.asarray(inputs["W3a"], np.float32)
    b3a = np.asarray(inputs["b3a"], np.float32)
    W3b = np.asarray(inputs["W3b"], np.float32)
    b3b = np.asarray(inputs["b3b"], np.float32)
    Wp1 = np.asarray(inputs["Wp1"], np.float32)
    bp1 = np.asarray(inputs["bp1"], np.float32)
    Wp2 = np.asarray(inputs["Wp2"], np.float32)
    bp2 = np.asarray(inputs["bp2"], np.float32)

    n = x2.shape[0]
    assert n == 50000 and src.shape == (R, 200000)
    ed = dec_src.shape[0]
    d3 = x3.shape[1]

    # per-edge weights
    ns_arr = np.stack([_deg_norm(src[r], n) for r in range(R)])
    nd_arr = np.stack([_deg_norm(dst[r], n) for r in range(R)])
    src_f = src.ravel()
    dst_f = dst.ravel()
    rel_f = np.repeat(np.arange(R, dtype=np.int64), src.shape[1])
    w_f = (ns_arr[rel_f, src_f] * nd_arr[rel_f, dst_f]).astype(np.float32)
    tile_f = dst_f >> 7
    half_f = (src_f >= HALF).astype(np.int64)

    cnt = np.bincount((tile_f * R + rel_f) * 2 + half_f,
                      minlength=NT * R * 2).reshape(NT, R, 2)
    # slots per (rel, half): tight multiple of 64; ORD=3 chunk-straddle
    # structure requires 256 < SL <= 384.
    SL = max(320, -(-int(cnt.max()) // 64) * 64)
    assert SL <= 384, f"group max {cnt.max()} exceeds SL ceiling"
    ORD = 3

    # rank of each edge within its (tile, rel, half) group
    key = (tile_f * R + rel_f) * 2 + half_f
    order = np.lexsort((half_f, rel_f, tile_f))
    ko = key[order]
    counts_flat = np.bincount(ko, minlength=NT * R * 2)
    starts = np.zeros(NT * R * 2, np.int64)
    np.cumsum(counts_flat[:-1], out=starts[1:])
    rank = np.empty_like(ko)
    rank[:] = np.arange(len(ko)) - starts[ko]
    so, do_, ro, ho, wo = (a[order] for a in (src_f, dst_f, rel_f, half_f, w_f))

    # idx values (relative to half table) and slot assignment
    ival = (so - ho * HALF).astype(np.int64)
    ia_cube = np.zeros((NT, 2, R * SL), np.int16)
    ia_cube[tile_f[order], ho, ro * SL + rank] = ival.astype(np.int16)

    # host-built one-hot scatter matrices S, per (half, rel, straddle-ord):
    # chunk c covers half-slots [128c, 128c+128); rel r occupies
    # [SL*r, SL*r+SL) -> ords 0..2 relative to c0 = (SL*r)//128.
    slot_ih = SL * ro + rank
    chunk_e = slot_ih >> 7
    row_e = slot_ih & 127
    ord_e = chunk_e - (SL * ro) // P
    S6 = np.zeros((NT, 2, R, ORD, P, P), np.float32)
    S6[tile_f[order], ho, ro, ord_e, row_e, do_ & 127] = wo
    smat_all = S6.reshape(NT, 2 * R * ORD, P, P).astype(ml_dtypes.bfloat16)
    del S6

    # x tables (bf16)
    xcat = np.zeros((NPAD, XC), np.float32)
    xcat[:n, :D2] = x2
    xcat[:n, D2:D2 + d3] = x3
    xcat = xcat.astype(ml_dtypes.bfloat16)
    xA_h = np.ascontiguousarray(xcat[:HALF])
    xB_h = np.ascontiguousarray(xcat[HALF:])

    # layer-A weights [R*NSL, 128, 256]
    WAp = np.zeros((R, NSL, P, 256), np.float32)
    for r in range(R):
        WAp[r, 0] = W2a[r, 0:P]
        WAp[r, 1] = W2a[r, P:2 * P]
        for k in range(2, NSL):
            f0 = k * P - D2
            rows = min(d3 - f0, P) if f0 < d3 else 0
            if rows > 0:
                WAp[r, k, :rows] = W3a[r, f0:f0 + rows]
    WAp = WAp.reshape(R * NSL, P, 256).astype(ml_dtypes.bfloat16)

    # decoder folding
    M = Wp1 @ Wp2
    A2, A3, B2, B3 = M[0:128], M[128:256], M[256:384], M[384:512]
    WMcat = np.zeros((512, P), np.float32)
    for r in range(R):
        WMcat[0:256, r * 16:(r + 1) * 16] = W2b[r] @ np.concatenate([A2, B2], axis=1)
        WMcat[256:512, r * 16:(r + 1) * 16] = W3b[r] @ np.concatenate([A3, B3], axis=1)
    WMp = WMcat.reshape(4, P, P).astype(ml_dtypes.bfloat16)
    c_total = (b2b.sum(0) @ np.concatenate([A2, B2], axis=1)
               + b3b.sum(0) @ np.concatenate([A3, B3], axis=1))
    c_total = c_total[0:8] + c_total[8:16] + bp1 @ Wp2 + bp2
    uvb_rep = np.tile(np.concatenate([np.zeros(8, np.float32),
                                      c_total.astype(np.float32)]), (P, 1))
    bias_rep = np.tile(np.concatenate([b2a.sum(0), b3a.sum(0)]).astype(np.float32), (P, 1))

    # decoder edges
    epc = -(-ed // NC)                       # 50000
    nb = -(-epc // DB)                       # 13
    in_maps = []
    for c in range(NC):
        tsl = slice(c * TPC, (c + 1) * TPC)
        ia_c = _rep128(_wrap16(ia_cube[tsl].reshape(TPC * 2, 8 * SL)))
        smat_c = np.ascontiguousarray(
            smat_all[tsl].transpose(0, 2, 1, 3).reshape(TPC, P, 2 * R * ORD * P))

        e0 = c * epc
        s_pad = np.zeros(nb * DB, np.int64)
        d_pad = np.zeros(nb * DB, np.int64)
        seg = slice(e0, min(e0 + epc, ed))
        ln = seg.stop - seg.start
        s_pad[:ln] = dec_src[seg]
        d_pad[:ln] = dec_dst[seg]
        di = np.concatenate([(s_pad >> 2).reshape(nb, DB),
                             (d_pad >> 2).reshape(nb, DB)], axis=1).astype(np.int16)
        di_c = _rep128(_wrap16(di))          # [128, nb*512]
        mu = (s_pad & 3).astype(np.float32).reshape(nb, 32, P).transpose(0, 2, 1)
        mv = (d_pad & 3).astype(np.float32).reshape(nb, 32, P).transpose(0, 2, 1)
        # [nb, 128, 32] -> repeat 8 -> [nb, 128, 256]
        mu = np.repeat(mu, 8, axis=2)
        mv = np.repeat(mv, 8, axis=2)
        # one-hot select masks per quarter: [nb, 8, 128, 256]
        em = np.stack([(mu == b4).astype(np.float32) for b4 in range(4)]
                      + [(mv == b4).astype(np.float32) for b4 in range(4)], axis=1)
        mk = np.ascontiguousarray(
            em.transpose(2, 0, 1, 3).reshape(P, nb * 8 * 256))
        in_maps.append(dict(
            xA=xA_h, xB=xB_h, WA=WAp, WM=WMp, bias_rep=bias_rep,
            uvb_rep=uvb_rep,
            idx_a=ia_c, smat=smat_c,
            idx_d=di_c, msk_d=mk,
        ))
    dims = dict(SL=SL, nb=nb, epc=epc, ed=ed)
    return in_maps, dims


_CACHE = {}


def kernel(**inputs):
    in_maps, dims = _prep(inputs)
    key = (dims["SL"], dims["nb"])
    nc = _CACHE.get(key)
    if nc is None:
        nc = _build(dims)
        _CACHE[key] = nc
    res = run_bass_kernel_spmd(nc, in_maps, list(range(NC)))
    epc, ed = dims["epc"], dims["ed"]
    out = np.concatenate(
        [res.results[c]["score_out"][:min(epc, ed - c * epc)] for c in range(NC)], axis=0)
    return np.ascontiguousarray(out.astype(np.float32))


if __name__ == "__main__":
    pass



# revision 30
# speedup vs baseline: 1.9315x; 1.0033x over previous
"""Trainium2 Bass kernel for the 2-layer hetero-GCN + linear edge decoder.

Math restructuring (exact, up to fp reassociation):
  hetero_conv: out = sum_r nd_r*(A_r @ (ns_r*x)) @ W_r + sum_r b_r
    -> per-edge weight w_e = ns_r[src]*nd_r[dst] folded into a one-hot
       scatter matrix S; aggregation z_r^T = g^T @ S runs directly
       transposed on the TensorEngine; the W_r matmul happens once per
       dst-tile.
  decoder has NO nonlinearity between Wp1 and Wp2, so
    score[e] = u[src_e] + v[dst_e] + c,  u = feat @ (Wp1@Wp2)[:256],
    v = feat @ (Wp1@Wp2)[256:], folded into the layer-B weight matmul
    (m = h @ WMcat, 16 useful cols per relation).

Perf structure (bottleneck = GpSimd Q7 descriptor generation for
dma_gather at ~8-9 ns/row, serial on the engine):
  - S matrices and decoder select masks are HOST-precomputed and DMA'd
    (one batched [128, 48*128] bf16 load per tile, shared by both conv
    layers) instead of built per chunk on the Vector engine.
  - Slot budget per (rel, half) is SL (tight data-max rounded to 64,
    256 < SL <= 384) rather than ceil-to-128: relation regions straddle
    128-slot chunk boundaries, handled by per-(rel, ord) S matrices
    whose rows outside the relation's range are zero.  This keeps the
    SPMD program uniform while cutting gather descriptors ~17%.
  - One merged gather per (tile, half) (single_packet=False is required
    for >64 descriptors/engine), identical idx arrays for both layers.
  - 5 gather buffers so Q7 descgen for tile t+1 never waits on tile
    t-1's PE consumption.

All matmuls and gathered tables run in bf16; PSUM accumulates fp32.

Sharding: dst-nodes partitioned into 128-node tiles, 49 tiles/core x 8
cores; x tables replicated; AllGather of the per-node message table m
([npad,128] bf16) and of the tiny (u,v) table; decoder edges sharded
evenly.
"""
import os
import sys

for _p in ("/opt/trn_rl_repo", "/root/.axon_site/_ro/trn_rl_repo"):
    if os.path.isdir(_p) and _p not in sys.path:
        sys.path.append(_p)

import numpy as np
import ml_dtypes

import concourse.bass as bass
import concourse.bacc as bacc
import concourse.mybir as mybir
import concourse.tile as tile
from concourse.bass_utils import run_bass_kernel_spmd
from concourse.masks import make_identity

P = 128
NC = 8
R = 8
F32 = mybir.dt.float32
BF16 = mybir.dt.bfloat16
I16 = mybir.dt.int16
I32 = mybir.dt.int32

DYN_COUNT = os.environ.get("K_DYN_COUNT", "0") == "1"

NPAD = 50176          # 392 tiles of 128
TPC = NPAD // (P * NC)  # 49 tiles per core
NT = NPAD // P        # 392
HALF = NPAD // 2      # 25088, split point for int16 gather indices
XC = 640              # padded x row (bf16 -> 1280B, mult of 256B)
NSL = 5               # 640 / 128 k-slices
D2 = 256
DB = 4096             # decoder edges per block


def _deg_norm(idx, n):
    deg = np.bincount(idx, minlength=n).astype(np.float32)
    out = np.zeros(n, np.float32)
    nz = deg > 0
    out[nz] = 1.0 / np.sqrt(np.maximum(deg[nz], 1.0))
    return out


def _wrap16(a):
    # [G, L] -> [16, G*L/16]: idx i of each group -> [i%16, group*L/16 + i//16]
    G, L = a.shape
    return a.reshape(G, L // 16, 16).transpose(2, 0, 1).reshape(16, G * (L // 16))


def _rep128(a16):
    return np.ascontiguousarray(np.tile(a16, (8, 1)))


def _build(dims):
    SL = dims["SL"]             # slots per (rel, half); 256 < SL <= 384
    ORD = 3                     # chunks touched per (rel, half)
    CHH = 8 * SL // P           # chunks per half
    NCHT = 16 * ORD             # S matrices per tile
    nb = dims["nb"]
    tpc = TPC
    ach = 8 * SL // 16          # idx cols per half

    nc = bacc.Bacc("TRN2", target_bir_lowering=False, debug=False)

    xA = nc.declare_dram_parameter("xA", [HALF, XC], BF16, isOutput=False)
    xB = nc.declare_dram_parameter("xB", [NPAD - HALF, XC], BF16, isOutput=False)
    WA = nc.declare_dram_parameter("WA", [R * NSL, P, 256], BF16, isOutput=False)
    WM = nc.declare_dram_parameter("WM", [4, P, P], BF16, isOutput=False)
    bias_rep = nc.declare_dram_parameter("bias_rep", [P, 512], F32, isOutput=False)
    uvb_rep = nc.declare_dram_parameter("uvb_rep", [P, 16], F32, isOutput=False)
    idx_a = nc.declare_dram_parameter("idx_a", [P, tpc * 2 * ach], I16, isOutput=False)
    smat = nc.declare_dram_parameter(
        "smat", [tpc, P, NCHT * P], BF16, isOutput=False)
    idx_d = nc.declare_dram_parameter("idx_d", [P, nb * 512], I16, isOutput=False)
    msk_d = nc.declare_dram_parameter("msk_d", [P, nb * 8 * 256], F32, isOutput=False)
    score_out = nc.declare_dram_parameter("score_out", [nb * DB, 8], F32, isOutput=True)

    AL = mybir.AluOpType

    with tile.TileContext(nc) as tc:
        with (
            tc.tile_pool(name="cpool", bufs=1) as cp,
            tc.tile_pool(name="dram", bufs=1, space="DRAM") as dp,
        ):
            m_shard = dp.tile([tpc * P, P], BF16)
            m_full = dp.tile([NPAD, P], BF16, addr_space="Shared")
            uv_shard = dp.tile([tpc * P, 16], F32)
            uv_full = dp.tile([NPAD // 4, 64], F32, addr_space="Shared")

            ident = cp.tile([P, P], BF16)
            make_identity(nc, ident[:])
            bias_t = cp.tile([P, 512], F32)
            nc.sync.dma_start(out=bias_t[:], in_=bias_rep[:, :])
            uvb_t = cp.tile([P, 16], F32)
            nc.sync.dma_start(out=uvb_t[:], in_=uvb_rep[:, :])
            wa_t = []
            for i in range(R * NSL):
                w_i = cp.tile([P, 256], BF16, tag=f"wa{i}")
                nc.sync.dma_start(out=w_i[:], in_=WA[i, :, :])
                wa_t.append(w_i)
            wm_t = []
            for k in range(4):
                w_k = cp.tile([P, P], BF16, tag=f"wm{k}")
                nc.sync.dma_start(out=w_k[:], in_=WM[k, :, :])
                wm_t.append(w_k)

            # ---------------- layer A ----------------
            GBUFS = 5
            with (
                tc.tile_pool(name="mpool", bufs=2) as mp,
                tc.tile_pool(name="gpool", bufs=GBUFS) as ga,
                tc.tile_pool(name="spool", bufs=2) as sp,
                tc.tile_pool(name="zpool", bufs=3) as zp,
                tc.tile_pool(name="pszpool", bufs=2, space="PSUM") as psz,
                tc.tile_pool(name="pshpool", bufs=1, space="PSUM") as psh,
                tc.tile_pool(name="pstpool", bufs=2, space="PSUM") as pst,
                tc.tile_pool(name="psmpool", bufs=1, space="PSUM") as psm,
            ):
                for t in range(tpc):
                    ia_t = mp.tile([P, 2 * ach], I16, tag="ia")
                    nc.sync.dma_start(out=ia_t[:], in_=idx_a[:, t * 2 * ach:(t + 1) * 2 * ach])
                    s_all = sp.tile([P, NCHT * P], BF16, tag="sall")
                    nc.sync.dma_start(out=s_all[:], in_=smat[t, :, :])
                    gt = []
                    for h in (0, 1):   # one merged gather per half
                        g = ga.tile([P, CHH, XC], BF16, tag="g")
                        nc.gpsimd.dma_gather(
                            out_ap=g[:], in_ap=(xA if h == 0 else xB)[:, :],
                            idxs_ap=ia_t[:, h * ach:(h + 1) * ach],
                            num_idxs=8 * SL, num_idxs_reg=8 * SL,
                            elem_size=XC, single_packet=False)
                        gt.append(g)
                    h2 = psh.tile([P, 256], F32, tag="h2")
                    h3 = psh.tile([P, 256], F32, tag="h3")
                    for r in range(R):
                        seq = []
                        for h in (0, 1):
                            for o in range(ORD):
                                lc = (SL * r) // P + o
                                seq.append((gt[h], lc, (h * 8 + r) * ORD + o))
                        for k in range(NSL):
                            ztk = psz.tile([P, P], F32, tag="ztk")
                            for i, (gtile, lc, gch) in enumerate(seq):
                                nc.tensor.matmul(
                                    ztk[:], gtile[:, lc, k * P:(k + 1) * P],
                                    s_all[:, gch * P:(gch + 1) * P],
                                    start=(i == 0), stop=(i == 2 * ORD - 1))
                            zsk = zp.tile([P, P], BF16, tag="zsk")
                            nc.scalar.copy(out=zsk[:], in_=ztk[:])
                            nc.tensor.matmul(
                                (h2 if k < 2 else h3)[:],
                                zsk[:], wa_t[r * NSL + k][:],
                                start=(r == 0 and k in (0, 2)),
                                stop=(r == R - 1 and k in (1, NSL - 1)))
                    hsb = zp.tile([P, 512], BF16, tag="hsb")
                    nc.vector.tensor_tensor(out=hsb[:, 0:256], in0=h2[:],
                                            in1=bias_t[:, 0:256], op=AL.add)
                    nc.vector.tensor_tensor(out=hsb[:, 256:512], in0=h3[:],
                                            in1=bias_t[:, 256:512], op=AL.add)
                    nc.vector.tensor_scalar_max(out=hsb[:], in0=hsb[:], scalar1=0.0)
                    ht4 = zp.tile([P, 4, P], BF16, tag="ht4")
                    for k in range(4):
                        tp = pst.tile([P, P], BF16, tag="tp")
                        nc.tensor.transpose(out=tp[:], in_=hsb[:, k * P:(k + 1) * P],
                                            identity=ident[:])
                        nc.scalar.copy(out=ht4[:, k, :], in_=tp[:])
                    mps = psm.tile([P, P], F32, tag="mps")
                    for k in range(4):
                        nc.tensor.matmul(mps[:], ht4[:, k, :], wm_t[k][:],
                                         start=(k == 0), stop=(k == 3))
                    msb = zp.tile([P, P], BF16, tag="msb")
                    nc.vector.tensor_copy(out=msb[:], in_=mps[:])
                    nc.sync.dma_start(out=m_shard[t * P:(t + 1) * P, :], in_=msb[:])

            nc.gpsimd.collective_compute(
                "AllGather", mybir.AluOpType.bypass,
                replica_groups=[list(range(NC))],
                ins=[m_shard[:, :]], outs=[m_full[:, :]])

            # ---------------- layer B ----------------
            GBBUFS = 4
            with (
                tc.tile_pool(name="mpoolb", bufs=2) as mp,
                tc.tile_pool(name="gpoolb", bufs=GBBUFS) as gb_p,
                tc.tile_pool(name="spoolb", bufs=2) as sp,
                tc.tile_pool(name="zpoolb", bufs=2) as zp,
                tc.tile_pool(name="psupool", bufs=2, space="PSUM") as psu,
            ):
                for t in range(tpc):
                    ib_t = mp.tile([P, 2 * ach], I16, tag="ib")
                    nc.sync.dma_start(out=ib_t[:], in_=idx_a[:, t * 2 * ach:(t + 1) * 2 * ach])
                    s_all = sp.tile([P, NCHT * P], BF16, tag="sallb")
                    nc.sync.dma_start(out=s_all[:], in_=smat[t, :, :])
                    gbt = []
                    for h in (0, 1):
                        g = gb_p.tile([P, CHH, P], BF16, tag="gb")
                        nc.gpsimd.dma_gather(
                            out_ap=g[:],
                            in_ap=(m_full[0:HALF, :] if h == 0 else m_full[HALF:NPAD, :]),
                            idxs_ap=ib_t[:, h * ach:(h + 1) * ach],
                            num_idxs=8 * SL, num_idxs_reg=8 * SL, elem_size=P,
                            single_packet=False)
                        gbt.append(g)
                    uvp = psu.tile([P, 16], F32, tag="uvp")
                    for h in (0, 1):
                        for r in range(R):
                            for o in range(ORD):
                                gch = (h * 8 + r) * ORD + o
                                nc.tensor.matmul(
                                    uvp[:], s_all[:, gch * P:(gch + 1) * P],
                                    gbt[h][:, (SL * r) // P + o, r * 16:(r + 1) * 16],
                                    start=(gch == 0), stop=(gch == NCHT - 1))
                    uvsb = zp.tile([P, 16], F32, tag="uvsb")
                    nc.vector.tensor_tensor(out=uvsb[:], in0=uvp[:], in1=uvb_t[:],
                                            op=AL.add)
                    nc.sync.dma_start(out=uv_shard[t * P:(t + 1) * P, :], in_=uvsb[:])

            nc.gpsimd.collective_compute(
                "AllGather", mybir.AluOpType.bypass,
                replica_groups=[list(range(NC))],
                ins=[uv_shard[:, :]], outs=[uv_full[:, :]])

            # ---------------- decoder ----------------
            with (
                tc.tile_pool(name="dmp", bufs=2) as mp,
                tc.tile_pool(name="dgp", bufs=2) as gp,
                tc.tile_pool(name="dvp", bufs=2) as vp,
            ):
                sview = score_out.ap().rearrange("(B j p) d -> B p j d", p=P, j=32)
                for b in range(nb):
                    id_t = mp.tile([P, 512], I16, tag="id")
                    nc.sync.dma_start(out=id_t[:], in_=idx_d[:, b * 512:(b + 1) * 512])
                    mk_t = mp.tile([P, 8, 32, 8], F32, tag="mk")
                    nc.sync.dma_start(
                        out=mk_t[:],
                        in_=msk_d[:, b * 2048:(b + 1) * 2048])
                    gd = gp.tile([P, 64, 64], F32, tag="gd")
                    nc.gpsimd.dma_gather(
                        out_ap=gd[:, 0:32, :], in_ap=uv_full[:, :],
                        idxs_ap=id_t[:, 0:256],
                        num_idxs=4096, num_idxs_reg=4096, elem_size=64,
                        single_packet=False)
                    nc.gpsimd.dma_gather(
                        out_ap=gd[:, 32:64, :], in_ap=uv_full[:, :],
                        idxs_ap=id_t[:, 256:512],
                        num_idxs=4096, num_idxs_reg=4096, elem_size=64,
                        single_packet=False)
                    acc = vp.tile([P, 32, 8], F32, tag="acc")
                    accv = vp.tile([P, 32, 8], F32, tag="accv")
                    for b4 in range(4):
                        if b4 == 0:
                            nc.vector.tensor_tensor(
                                out=acc[:], in0=mk_t[:, 0, :, :],
                                in1=gd[:, 0:32, 0:8], op=AL.mult)
                        else:
                            eq = vp.tile([P, 32, 8], F32, tag="eq")
                            nc.vector.tensor_tensor(
                                out=eq[:], in0=mk_t[:, b4, :, :],
                                in1=gd[:, 0:32, b4 * 16:b4 * 16 + 8], op=AL.mult)
                            nc.vector.tensor_tensor(out=acc[:], in0=acc[:], in1=eq[:],
                                                    op=AL.add)
                    for b4 in range(4):
                        if b4 == 0:
                            nc.vector.tensor_tensor(
                                out=accv[:], in0=mk_t[:, 4, :, :],
                                in1=gd[:, 32:64, 8:16], op=AL.mult)
                        else:
                            eq = vp.tile([P, 32, 8], F32, tag="eq")
                            nc.vector.tensor_tensor(
                                out=eq[:], in0=mk_t[:, 4 + b4, :, :],
                                in1=gd[:, 32:64, b4 * 16 + 8:b4 * 16 + 16], op=AL.mult)
                            nc.vector.tensor_tensor(out=accv[:], in0=accv[:], in1=eq[:],
                                                    op=AL.add)
                    nc.vector.tensor_tensor(out=acc[:], in0=acc[:], in1=accv[:],
                                            op=AL.add)
                    nc.sync.dma_start(out=sview[b], in_=acc[:])
    nc.finalize()
    return nc


def _prep(inputs):
    x2 = np.asarray(inputs["node2_features"], np.float32)
    x3 = np.asarray(inputs["mpnn_features"], np.float32)
    src = np.asarray(inputs["src"]).astype(np.int64)
    dst = np.asarray(inputs["dst"]).astype(np.int64)
    dec_src = np.asarray(inputs["dec_src"]).astype(np.int64)
    dec_dst = np.asarray(inputs["dec_dst"]).astype(np.int64)
    W2a = np.asarray(inputs["W2a"], np.float32)
    b2a = np.asarray(inputs["b2a"], np.float32)
    W2b = np.asarray(inputs["W2b"], np.float32)
    b2b = np.asarray(inputs["b2b"], np.float32)
    W3a = np.asarray(inputs["W3a"], np.float32)
    b3a = np.asarray(inputs["b3a"], np.float32)
    W3b = np.asarray(inputs["W3b"], np.float32)
    b3b = np.asarray(inputs["b3b"], np.float32)
    Wp1 = np.asarray(inputs["Wp1"], np.float32)
    bp1 = np.asarray(inputs["bp1"], np.float32)
    Wp2 = np.asarray(inputs["Wp2"], np.float32)
    bp2 = np.asarray(inputs["bp2"], np.float32)

    n = x2.shape[0]
    assert n == 50000 and src.shape == (R, 200000)
    ed = dec_src.shape[0]
    d3 = x3.shape[1]

    # per-edge weights
    ns_arr = np.stack([_deg_norm(src[r], n) for r in range(R)])
    nd_arr = np.stack([_deg_norm(dst[r], n) for r in range(R)])
    src_f = src.ravel()
    dst_f = dst.ravel()
    rel_f = np.repeat(np.arange(R, dtype=np.int64), src.shape[1])
    w_f = (ns_arr[rel_f, src_f] * nd_arr[rel_f, dst_f]).astype(np.float32)
    tile_f = dst_f >> 7
    half_f = (src_f >= HALF).astype(np.int64)

    cnt = np.bincount((tile_f * R + rel_f) * 2 + half_f,
                      minlength=NT * R * 2).reshape(NT, R, 2)
    # slots per (rel, half): tight multiple of 64; ORD=3 chunk-straddle
    # structure requires 256 < SL <= 384.
    SL = max(320, -(-int(cnt.max()) // 64) * 64)
    assert SL <= 384, f"group max {cnt.max()} exceeds SL ceiling"
    ORD = 3

    # rank of each edge within its (tile, rel, half) group
    key = (tile_f * R + rel_f) * 2 + half_f
    order = np.lexsort((half_f, rel_f, tile_f))
    ko = key[order]
    counts_flat = np.bincount(ko, minlength=NT * R * 2)
    starts = np.zeros(NT * R * 2, np.int64)
    np.cumsum(counts_flat[:-1], out=starts[1:])
    rank = np.empty_like(ko)
    rank[:] = np.arange(len(ko)) - starts[ko]
    so, do_, ro, ho, wo = (a[order] for a in (src_f, dst_f, rel_f, half_f, w_f))

    # idx values (relative to half table) and slot assignment
    ival = (so - ho * HALF).astype(np.int64)
    ia_cube = np.zeros((NT, 2, R * SL), np.int16)
    ia_cube[tile_f[order], ho, ro * SL + rank] = ival.astype(np.int16)

    # host-built one-hot scatter matrices S, per (half, rel, straddle-ord):
    # chunk c covers half-slots [128c, 128c+128); rel r occupies
    # [SL*r, SL*r+SL) -> ords 0..2 relative to c0 = (SL*r)//128.
    slot_ih = SL * ro + rank
    chunk_e = slot_ih >> 7
    row_e = slot_ih & 127
    ord_e = chunk_e - (SL * ro) // P
    S6 = np.zeros((NT, 2, R, ORD, P, P), np.float32)
    S6[tile_f[order], ho, ro, ord_e, row_e, do_ & 127] = wo
    smat_all = S6.reshape(NT, 2 * R * ORD, P, P).astype(ml_dtypes.bfloat16)
    del S6

    # x tables (bf16)
    xcat = np.zeros((NPAD, XC), np.float32)
    xcat[:n, :D2] = x2
    xcat[:n, D2:D2 + d3] = x3
    xcat = xcat.astype(ml_dtypes.bfloat16)
    xA_h = np.ascontiguousarray(xcat[:HALF])
    xB_h = np.ascontiguousarray(xcat[HALF:])

    # layer-A weights [R*NSL, 128, 256]
    WAp = np.zeros((R, NSL, P, 256), np.float32)
    for r in range(R):
        WAp[r, 0] = W2a[r, 0:P]
        WAp[r, 1] = W2a[r, P:2 * P]
        for k in range(2, NSL):
            f0 = k * P - D2
            rows = min(d3 - f0, P) if f0 < d3 else 0
            if rows > 0:
                WAp[r, k, :rows] = W3a[r, f0:f0 + rows]
    WAp = WAp.reshape(R * NSL, P, 256).astype(ml_dtypes.bfloat16)

    # decoder folding
    M = Wp1 @ Wp2
    A2, A3, B2, B3 = M[0:128], M[128:256], M[256:384], M[384:512]
    WMcat = np.zeros((512, P), np.float32)
    for r in range(R):
        WMcat[0:256, r * 16:(r + 1) * 16] = W2b[r] @ np.concatenate([A2, B2], axis=1)
        WMcat[256:512, r * 16:(r + 1) * 16] = W3b[r] @ np.concatenate([A3, B3], axis=1)
    WMp = WMcat.reshape(4, P, P).astype(ml_dtypes.bfloat16)
    c_total = (b2b.sum(0) @ np.concatenate([A2, B2], axis=1)
               + b3b.sum(0) @ np.concatenate([A3, B3], axis=1))
    c_total = c_total[0:8] + c_total[8:16] + bp1 @ Wp2 + bp2
    uvb_rep = np.tile(np.concatenate([np.zeros(8, np.float32),
                                      c_total.astype(np.float32)]), (P, 1))
    bias_rep = np.tile(np.concatenate([b2a.sum(0), b3a.sum(0)]).astype(np.float32), (P, 1))

    # decoder edges
    epc = -(-ed // NC)                       # 50000
    nb = -(-epc // DB)                       # 13
    in_maps = []
    for c in range(NC):
        tsl = slice(c * TPC, (c + 1) * TPC)
        ia_c = _rep128(_wrap16(ia_cube[tsl].reshape(TPC * 2, 8 * SL)))
        smat_c = np.ascontiguousarray(
            smat_all[tsl].transpose(0, 2, 1, 3).reshape(TPC, P, 2 * R * ORD * P))

        e0 = c * epc
        s_pad = np.zeros(nb * DB, np.int64)
        d_pad = np.zeros(nb * DB, np.int64)
        seg = slice(e0, min(e0 + epc, ed))
        ln = seg.stop - seg.start
        s_pad[:ln] = dec_src[seg]
        d_pad[:ln] = dec_dst[seg]
        di = np.concatenate([(s_pad >> 2).reshape(nb, DB),
                             (d_pad >> 2).reshape(nb, DB)], axis=1).astype(np.int16)
        di_c = _rep128(_wrap16(di))          # [128, nb*512]
        mu = (s_pad & 3).astype(np.float32).reshape(nb, 32, P).transpose(0, 2, 1)
        mv = (d_pad & 3).astype(np.float32).reshape(nb, 32, P).transpose(0, 2, 1)
        # [nb, 128, 32] -> repeat 8 -> [nb, 128, 256]
        mu = np.repeat(mu, 8, axis=2)
        mv = np.repeat(mv, 8, axis=2)
        # one-hot select masks per quarter: [nb, 8, 128, 256]
        em = np.stack([(mu == b4).astype(np.float32) for b4 in range(4)]
                      + [(mv == b4).astype(np.float32) for b4 in range(4)], axis=1)
        mk = np.ascontiguousarray(
            em.transpose(2, 0, 1, 3).reshape(P, nb * 8 * 256))
        in_maps.append(dict(
            xA=xA_h, xB=xB_h, WA=WAp, WM=WMp, bias_rep=bias_rep,
            uvb_rep=uvb_rep,
            idx_a=ia_c, smat=smat_c,
            idx_d=di_c, msk_d=mk,
        ))
    dims = dict(SL=SL, nb=nb, epc=epc, ed=ed)
    return in_maps, dims


_CACHE = {}


def kernel(**inputs):
    in_maps, dims = _prep(inputs)
    key = (dims["SL"], dims["nb"])
    nc = _CACHE.get(key)
    if nc is None:
        nc = _build(dims)
        _CACHE[key] = nc
    res = run_bass_kernel_spmd(nc, in_maps, list(range(NC)))
    epc, ed = dims["epc"], dims["ed"]
    out = np.concatenate(
        [res.results[c]["score_out"][:min(epc, ed - c * epc)] for c in range(NC)], axis=0)
    return np.ascontiguousarray(out.astype(np.float32))


if __name__ == "__main__":
    pass



# revision 38
# speedup vs baseline: 2.1493x; 1.1127x over previous
"""Trainium2 Bass kernel for the 2-layer hetero-GCN + linear edge decoder.

Math restructuring (exact, up to fp reassociation):
  hetero_conv: out = sum_r nd_r*(A_r @ (ns_r*x)) @ W_r + sum_r b_r
    -> per-edge weight w_e = ns_r[src]*nd_r[dst] folded into a one-hot
       scatter matrix S; aggregation z_r^T = g^T @ S runs directly
       transposed on the TensorEngine; the W_r matmul happens once per
       dst-tile.
  decoder has NO nonlinearity between Wp1 and Wp2, so
    score[e] = u[src_e] + v[dst_e] + c,  u = feat @ (Wp1@Wp2)[:256],
    v = feat @ (Wp1@Wp2)[256:], folded into the layer-B weight matmul
    (m = h @ WMcat, 16 useful cols per relation).

Perf structure (bottleneck = GpSimd Q7 descriptor generation for
dma_gather at ~8-9 ns/row, serial on the engine):
  - S matrices and decoder select masks are HOST-precomputed and DMA'd
    (one batched [128, 48*128] bf16 load per tile, shared by both conv
    layers) instead of built per chunk on the Vector engine.
  - Slot budget per (rel, half) is SL (tight data-max rounded to 64,
    256 < SL <= 384) rather than ceil-to-128: relation regions straddle
    128-slot chunk boundaries, handled by per-(rel, ord) S matrices
    whose rows outside the relation's range are zero.  This keeps the
    SPMD program uniform while cutting gather descriptors ~17%.
  - One merged gather per (tile, half) (single_packet=False is required
    for >64 descriptors/engine), identical idx arrays for both layers.
  - 5 gather buffers so Q7 descgen for tile t+1 never waits on tile
    t-1's PE consumption.

All matmuls and gathered tables run in bf16; PSUM accumulates fp32.

Sharding: dst-nodes partitioned into 128-node tiles, 49 tiles/core x 8
cores; x tables replicated; AllGather of the per-node message table m
([npad,128] bf16) and of the tiny (u,v) table; decoder edges sharded
evenly.
"""
import os
import sys

for _p in ("/opt/trn_rl_repo", "/root/.axon_site/_ro/trn_rl_repo"):
    if os.path.isdir(_p) and _p not in sys.path:
        sys.path.append(_p)

import numpy as np
import ml_dtypes

import concourse.bass as bass
import concourse.bacc as bacc
import concourse.mybir as mybir
import concourse.tile as tile
from concourse.bass_utils import run_bass_kernel_spmd
from concourse.masks import make_identity

P = 128
NC = 8
R = 8
F32 = mybir.dt.float32
BF16 = mybir.dt.bfloat16
I16 = mybir.dt.int16
I32 = mybir.dt.int32

DYN_COUNT = os.environ.get("K_DYN_COUNT", "0") == "1"

NPAD = 50176          # 392 tiles of 128
TPC = NPAD // (P * NC)  # 49 tiles per core
NT = NPAD // P        # 392
HALF = NPAD // 2      # 25088, split point for int16 gather indices
XC = 640              # padded x row (bf16 -> 1280B, mult of 256B)
NSL = 5               # 640 / 128 k-slices
D2 = 256
DB = 4096             # decoder edges per block


def _deg_norm(idx, n):
    deg = np.bincount(idx, minlength=n).astype(np.float32)
    out = np.zeros(n, np.float32)
    nz = deg > 0
    out[nz] = 1.0 / np.sqrt(np.maximum(deg[nz], 1.0))
    return out


def _wrap16(a):
    # [G, L] -> [16, G*L/16]: idx i of each group -> [i%16, group*L/16 + i//16]
    G, L = a.shape
    return a.reshape(G, L // 16, 16).transpose(2, 0, 1).reshape(16, G * (L // 16))


def _rep128(a16):
    return np.ascontiguousarray(np.tile(a16, (8, 1)))


def _build(dims):
    CHH = dims["CHH"]           # chunks per half (16 region + 2 overflow)
    ORD = 3                     # matmul entries per (rel, half)
    NCHT = 16 * ORD             # S matrices per tile
    nb = dims["nb"]
    tpc = TPC
    ach = CHH * P // 16         # idx cols per half

    nc = bacc.Bacc("TRN2", target_bir_lowering=False, debug=False)

    xA = nc.declare_dram_parameter("xA", [HALF, XC], BF16, isOutput=False)
    xB = nc.declare_dram_parameter("xB", [NPAD - HALF, XC], BF16, isOutput=False)
    WA = nc.declare_dram_parameter("WA", [R * NSL, P, 256], BF16, isOutput=False)
    WM = nc.declare_dram_parameter("WM", [4, P, P], BF16, isOutput=False)
    bias_rep = nc.declare_dram_parameter("bias_rep", [P, 512], F32, isOutput=False)
    uvb_rep = nc.declare_dram_parameter("uvb_rep", [P, 16], F32, isOutput=False)
    idx_a = nc.declare_dram_parameter("idx_a", [P, tpc * 2 * ach], I16, isOutput=False)
    smat = nc.declare_dram_parameter(
        "smat", [tpc, P, NCHT * P], BF16, isOutput=False)
    idx_d = nc.declare_dram_parameter("idx_d", [P, nb * 512], I16, isOutput=False)
    msk_d = nc.declare_dram_parameter("msk_d", [P, nb * 8 * 256], F32, isOutput=False)
    score_out = nc.declare_dram_parameter("score_out", [nb * DB, 8], F32, isOutput=True)

    AL = mybir.AluOpType

    with tile.TileContext(nc) as tc:
        with (
            tc.tile_pool(name="cpool", bufs=1) as cp,
            tc.tile_pool(name="dram", bufs=1, space="DRAM") as dp,
        ):
            m_shard = dp.tile([tpc * P, P], BF16)
            m_full = dp.tile([NPAD, P], BF16, addr_space="Shared")
            uv_shard = dp.tile([tpc * P, 16], F32)
            uv_full = dp.tile([NPAD // 4, 64], F32, addr_space="Shared")

            ident = cp.tile([P, P], BF16)
            make_identity(nc, ident[:])
            bias_t = cp.tile([P, 512], F32)
            nc.sync.dma_start(out=bias_t[:], in_=bias_rep[:, :])
            uvb_t = cp.tile([P, 16], F32)
            nc.sync.dma_start(out=uvb_t[:], in_=uvb_rep[:, :])
            wa_t = []
            for i in range(R * NSL):
                w_i = cp.tile([P, 256], BF16, tag=f"wa{i}")
                nc.sync.dma_start(out=w_i[:], in_=WA[i, :, :])
                wa_t.append(w_i)
            wm_t = []
            for k in range(4):
                w_k = cp.tile([P, P], BF16, tag=f"wm{k}")
                nc.sync.dma_start(out=w_k[:], in_=WM[k, :, :])
                wm_t.append(w_k)

            # ---------------- layer A ----------------
            GBUFS = 6
            with (
                tc.tile_pool(name="mpool", bufs=2) as mp,
                tc.tile_pool(name="gpool", bufs=GBUFS) as ga,
                tc.tile_pool(name="spool", bufs=2) as sp,
                tc.tile_pool(name="zpool", bufs=3) as zp,
                tc.tile_pool(name="pszpool", bufs=2, space="PSUM") as psz,
                tc.tile_pool(name="pshpool", bufs=1, space="PSUM") as psh,
                tc.tile_pool(name="pstpool", bufs=2, space="PSUM") as pst,
                tc.tile_pool(name="psmpool", bufs=1, space="PSUM") as psm,
            ):
                for t in range(tpc):
                    ia_t = mp.tile([P, 2 * ach], I16, tag="ia")
                    nc.sync.dma_start(out=ia_t[:], in_=idx_a[:, t * 2 * ach:(t + 1) * 2 * ach])
                    s_all = sp.tile([P, NCHT * P], BF16, tag="sall")
                    nc.sync.dma_start(out=s_all[:], in_=smat[t, :, :])
                    gt = []
                    for h in (0, 1):   # one merged gather per half
                        g = ga.tile([P, CHH, XC], BF16, tag="g")
                        nc.gpsimd.dma_gather(
                            out_ap=g[:], in_ap=(xA if h == 0 else xB)[:, :],
                            idxs_ap=ia_t[:, h * ach:(h + 1) * ach],
                            num_idxs=CHH * P, num_idxs_reg=CHH * P,
                            elem_size=XC, single_packet=False)
                        gt.append(g)
                    h2 = psh.tile([P, 256], F32, tag="h2")
                    h3 = psh.tile([P, 256], F32, tag="h3")
                    for r in range(R):
                        seq = []
                        for h in (0, 1):
                            for o in range(ORD):
                                lc = 2 * r + o if o < 2 else 16 + r // 4
                                seq.append((gt[h], lc, (h * 8 + r) * ORD + o))
                        for k in range(NSL):
                            ztk = psz.tile([P, P], F32, tag="ztk")
                            for i, (gtile, lc, gch) in enumerate(seq):
                                nc.tensor.matmul(
                                    ztk[:], gtile[:, lc, k * P:(k + 1) * P],
                                    s_all[:, gch * P:(gch + 1) * P],
                                    start=(i == 0), stop=(i == 2 * ORD - 1))
                            zsk = zp.tile([P, P], BF16, tag="zsk")
                            nc.scalar.copy(out=zsk[:], in_=ztk[:])
                            nc.tensor.matmul(
                                (h2 if k < 2 else h3)[:],
                                zsk[:], wa_t[r * NSL + k][:],
                                start=(r == 0 and k in (0, 2)),
                                stop=(r == R - 1 and k in (1, NSL - 1)))
                    hsb = zp.tile([P, 512], BF16, tag="hsb")
                    nc.vector.tensor_tensor(out=hsb[:, 0:256], in0=h2[:],
                                            in1=bias_t[:, 0:256], op=AL.add)
                    nc.vector.tensor_tensor(out=hsb[:, 256:512], in0=h3[:],
                                            in1=bias_t[:, 256:512], op=AL.add)
                    nc.vector.tensor_scalar_max(out=hsb[:], in0=hsb[:], scalar1=0.0)
                    ht4 = zp.tile([P, 4, P], BF16, tag="ht4")
                    for k in range(4):
                        tp = pst.tile([P, P], BF16, tag="tp")
                        nc.tensor.transpose(out=tp[:], in_=hsb[:, k * P:(k + 1) * P],
                                            identity=ident[:])
                        nc.scalar.copy(out=ht4[:, k, :], in_=tp[:])
                    mps = psm.tile([P, P], F32, tag="mps")
                    for k in range(4):
                        nc.tensor.matmul(mps[:], ht4[:, k, :], wm_t[k][:],
                                         start=(k == 0), stop=(k == 3))
                    msb = zp.tile([P, P], BF16, tag="msb")
                    nc.vector.tensor_copy(out=msb[:], in_=mps[:])
                    nc.sync.dma_start(out=m_shard[t * P:(t + 1) * P, :], in_=msb[:])

            nc.gpsimd.collective_compute(
                "AllGather", mybir.AluOpType.bypass,
                replica_groups=[list(range(NC))],
                ins=[m_shard[:, :]], outs=[m_full[:, :]])

            # ---------------- layer B ----------------
            GBBUFS = 4
            with (
                tc.tile_pool(name="mpoolb", bufs=2) as mp,
                tc.tile_pool(name="gpoolb", bufs=GBBUFS) as gb_p,
                tc.tile_pool(name="spoolb", bufs=2) as sp,
                tc.tile_pool(name="zpoolb", bufs=2) as zp,
                tc.tile_pool(name="psupool", bufs=2, space="PSUM") as psu,
            ):
                for t in range(tpc):
                    ib_t = mp.tile([P, 2 * ach], I16, tag="ib")
                    nc.sync.dma_start(out=ib_t[:], in_=idx_a[:, t * 2 * ach:(t + 1) * 2 * ach])
                    s_all = sp.tile([P, NCHT * P], BF16, tag="sallb")
                    nc.sync.dma_start(out=s_all[:], in_=smat[t, :, :])
                    gbt = []
                    for h in (0, 1):
                        g = gb_p.tile([P, CHH, P], BF16, tag="gb")
                        nc.gpsimd.dma_gather(
                            out_ap=g[:],
                            in_ap=(m_full[0:HALF, :] if h == 0 else m_full[HALF:NPAD, :]),
                            idxs_ap=ib_t[:, h * ach:(h + 1) * ach],
                            num_idxs=CHH * P, num_idxs_reg=CHH * P, elem_size=P,
                            single_packet=False)
                        gbt.append(g)
                    uvp = psu.tile([P, 16], F32, tag="uvp")
                    for h in (0, 1):
                        for r in range(R):
                            for o in range(ORD):
                                gch = (h * 8 + r) * ORD + o
                                lc = 2 * r + o if o < 2 else 16 + r // 4
                                nc.tensor.matmul(
                                    uvp[:], s_all[:, gch * P:(gch + 1) * P],
                                    gbt[h][:, lc, r * 16:(r + 1) * 16],
                                    start=(gch == 0), stop=(gch == NCHT - 1))
                    uvsb = zp.tile([P, 16], F32, tag="uvsb")
                    nc.vector.tensor_tensor(out=uvsb[:], in0=uvp[:], in1=uvb_t[:],
                                            op=AL.add)
                    nc.sync.dma_start(out=uv_shard[t * P:(t + 1) * P, :], in_=uvsb[:])

            nc.gpsimd.collective_compute(
                "AllGather", mybir.AluOpType.bypass,
                replica_groups=[list(range(NC))],
                ins=[uv_shard[:, :]], outs=[uv_full[:, :]])

            # ---------------- decoder ----------------
            with (
                tc.tile_pool(name="dmp", bufs=2) as mp,
                tc.tile_pool(name="dgp", bufs=2) as gp,
                tc.tile_pool(name="dvp", bufs=2) as vp,
            ):
                sview = score_out.ap().rearrange("(B j p) d -> B p j d", p=P, j=32)
                for b in range(nb):
                    id_t = mp.tile([P, 512], I16, tag="id")
                    nc.sync.dma_start(out=id_t[:], in_=idx_d[:, b * 512:(b + 1) * 512])
                    mk_t = mp.tile([P, 8, 32, 8], F32, tag="mk")
                    nc.sync.dma_start(
                        out=mk_t[:],
                        in_=msk_d[:, b * 2048:(b + 1) * 2048])
                    gd = gp.tile([P, 64, 64], F32, tag="gd")
                    nc.gpsimd.dma_gather(
                        out_ap=gd[:, 0:32, :], in_ap=uv_full[:, :],
                        idxs_ap=id_t[:, 0:256],
                        num_idxs=4096, num_idxs_reg=4096, elem_size=64,
                        single_packet=False)
                    nc.gpsimd.dma_gather(
                        out_ap=gd[:, 32:64, :], in_ap=uv_full[:, :],
                        idxs_ap=id_t[:, 256:512],
                        num_idxs=4096, num_idxs_reg=4096, elem_size=64,
                        single_packet=False)
                    acc = vp.tile([P, 32, 8], F32, tag="acc")
                    accv = vp.tile([P, 32, 8], F32, tag="accv")
                    for b4 in range(4):
                        if b4 == 0:
                            nc.vector.tensor_tensor(
                                out=acc[:], in0=mk_t[:, 0, :, :],
                                in1=gd[:, 0:32, 0:8], op=AL.mult)
                        else:
                            eq = vp.tile([P, 32, 8], F32, tag="eq")
                            nc.vector.tensor_tensor(
                                out=eq[:], in0=mk_t[:, b4, :, :],
                                in1=gd[:, 0:32, b4 * 16:b4 * 16 + 8], op=AL.mult)
                            nc.vector.tensor_tensor(out=acc[:], in0=acc[:], in1=eq[:],
                                                    op=AL.add)
                    for b4 in range(4):
                        if b4 == 0:
                            nc.vector.tensor_tensor(
                                out=accv[:], in0=mk_t[:, 4, :, :],
                                in1=gd[:, 32:64, 8:16], op=AL.mult)
                        else:
                            eq = vp.tile([P, 32, 8], F32, tag="eq")
                            nc.vector.tensor_tensor(
                                out=eq[:], in0=mk_t[:, 4 + b4, :, :],
                                in1=gd[:, 32:64, b4 * 16 + 8:b4 * 16 + 16], op=AL.mult)
                            nc.vector.tensor_tensor(out=accv[:], in0=accv[:], in1=eq[:],
                                                    op=AL.add)
                    nc.vector.tensor_tensor(out=acc[:], in0=acc[:], in1=accv[:],
                                            op=AL.add)
                    nc.sync.dma_start(out=sview[b], in_=acc[:])
    nc.finalize()
    return nc


def _prep(inputs):
    x2 = np.asarray(inputs["node2_features"], np.float32)
    x3 = np.asarray(inputs["mpnn_features"], np.float32)
    src = np.asarray(inputs["src"]).astype(np.int64)
    dst = np.asarray(inputs["dst"]).astype(np.int64)
    dec_src = np.asarray(inputs["dec_src"]).astype(np.int64)
    dec_dst = np.asarray(inputs["dec_dst"]).astype(np.int64)
    W2a = np.asarray(inputs["W2a"], np.float32)
    b2a = np.asarray(inputs["b2a"], np.float32)
    W2b = np.asarray(inputs["W2b"], np.float32)
    b2b = np.asarray(inputs["b2b"], np.float32)
    W3a = np.asarray(inputs["W3a"], np.float32)
    b3a = np.asarray(inputs["b3a"], np.float32)
    W3b = np.asarray(inputs["W3b"], np.float32)
    b3b = np.asarray(inputs["b3b"], np.float32)
    Wp1 = np.asarray(inputs["Wp1"], np.float32)
    bp1 = np.asarray(inputs["bp1"], np.float32)
    Wp2 = np.asarray(inputs["Wp2"], np.float32)
    bp2 = np.asarray(inputs["bp2"], np.float32)

    n = x2.shape[0]
    assert n == 50000 and src.shape == (R, 200000)
    ed = dec_src.shape[0]
    d3 = x3.shape[1]

    # per-edge weights
    ns_arr = np.stack([_deg_norm(src[r], n) for r in range(R)])
    nd_arr = np.stack([_deg_norm(dst[r], n) for r in range(R)])
    src_f = src.ravel()
    dst_f = dst.ravel()
    rel_f = np.repeat(np.arange(R, dtype=np.int64), src.shape[1])
    w_f = (ns_arr[rel_f, src_f] * nd_arr[rel_f, dst_f]).astype(np.float32)
    tile_f = dst_f >> 7
    half_f = (src_f >= HALF).astype(np.int64)

    cnt = np.bincount((tile_f * R + rel_f) * 2 + half_f,
                      minlength=NT * R * 2).reshape(NT, R, 2)
    # 256-slot (2-chunk) region per (rel, half) + two shared overflow
    # chunks per half: rels 0-3 spill to chunk 16, rels 4-7 to chunk 17.
    RS = 256
    ORD = 3
    CHH = 18
    ovg = np.maximum(cnt - RS, 0)
    for g2 in range(2):
        ovmax = int(ovg[:, 4 * g2:4 * g2 + 4, :].sum(axis=1).max())
        assert ovmax <= P, f"overflow group {g2} max {ovmax} exceeds 128"

    # rank of each edge within its (tile, rel, half) group
    key = (tile_f * R + rel_f) * 2 + half_f
    order = np.lexsort((half_f, rel_f, tile_f))
    ko = key[order]
    counts_flat = np.bincount(ko, minlength=NT * R * 2)
    starts = np.zeros(NT * R * 2, np.int64)
    np.cumsum(counts_flat[:-1], out=starts[1:])
    rank = np.empty_like(ko)
    rank[:] = np.arange(len(ko)) - starts[ko]
    so, do_, ro, ho, wo = (a[order] for a in (src_f, dst_f, rel_f, half_f, w_f))

    # slot assignment: regular slots RS*r + rank; overflow edges get
    # sequential slots in their half's (r//4) overflow chunk.
    reg = rank < RS
    rg = ro // 4
    okey = (tile_f[order] * 2 + ho) * 2 + rg
    ov_i = np.flatnonzero(~reg)
    oks = okey[ov_i]
    sor = np.argsort(oks, kind="stable")
    okss = oks[sor]
    pos = np.arange(len(okss))
    firsts = np.zeros(len(okss), np.int64)
    gs = np.r_[0, np.flatnonzero(np.diff(okss)) + 1]
    firsts[gs] = pos[gs]
    firsts = np.maximum.accumulate(firsts)
    ovpos = np.zeros(len(rank), np.int64)
    ovpos[ov_i[sor]] = pos - firsts
    slot_ih = np.where(reg, RS * ro + rank, 16 * P + P * rg + ovpos)
    assert slot_ih.max() < CHH * P
    row_e = slot_ih & 127
    ord_e = np.where(reg, rank >> 7, 2)

    # idx values (relative to half table)
    ival = (so - ho * HALF).astype(np.int64)
    ia_cube = np.zeros((NT, 2, CHH * P), np.int16)
    ia_cube[tile_f[order], ho, slot_ih] = ival.astype(np.int16)

    # host-built one-hot scatter matrices S per (half, rel, ord):
    # ord 0/1 = the rel's own chunks 2r/2r+1; ord 2 = the shared
    # overflow chunk (rows of other rels are zero).
    S6 = np.zeros((NT, 2, R, ORD, P, P), np.float32)
    S6[tile_f[order], ho, ro, ord_e, row_e, do_ & 127] = wo
    smat_all = S6.reshape(NT, 2 * R * ORD, P, P).astype(ml_dtypes.bfloat16)
    del S6

    # x tables (bf16)
    xcat = np.zeros((NPAD, XC), np.float32)
    xcat[:n, :D2] = x2
    xcat[:n, D2:D2 + d3] = x3
    xcat = xcat.astype(ml_dtypes.bfloat16)
    xA_h = np.ascontiguousarray(xcat[:HALF])
    xB_h = np.ascontiguousarray(xcat[HALF:])

    # layer-A weights [R*NSL, 128, 256]
    WAp = np.zeros((R, NSL, P, 256), np.float32)
    for r in range(R):
        WAp[r, 0] = W2a[r, 0:P]
        WAp[r, 1] = W2a[r, P:2 * P]
        for k in range(2, NSL):
            f0 = k * P - D2
            rows = min(d3 - f0, P) if f0 < d3 else 0
            if rows > 0:
                WAp[r, k, :rows] = W3a[r, f0:f0 + rows]
    WAp = WAp.reshape(R * NSL, P, 256).astype(ml_dtypes.bfloat16)

    # decoder folding
    M = Wp1 @ Wp2
    A2, A3, B2, B3 = M[0:128], M[128:256], M[256:384], M[384:512]
    WMcat = np.zeros((512, P), np.float32)
    for r in range(R):
        WMcat[0:256, r * 16:(r + 1) * 16] = W2b[r] @ np.concatenate([A2, B2], axis=1)
        WMcat[256:512, r * 16:(r + 1) * 16] = W3b[r] @ np.concatenate([A3, B3], axis=1)
    WMp = WMcat.reshape(4, P, P).astype(ml_dtypes.bfloat16)
    c_total = (b2b.sum(0) @ np.concatenate([A2, B2], axis=1)
               + b3b.sum(0) @ np.concatenate([A3, B3], axis=1))
    c_total = c_total[0:8] + c_total[8:16] + bp1 @ Wp2 + bp2
    uvb_rep = np.tile(np.concatenate([np.zeros(8, np.float32),
                                      c_total.astype(np.float32)]), (P, 1))
    bias_rep = np.tile(np.concatenate([b2a.sum(0), b3a.sum(0)]).astype(np.float32), (P, 1))

    # decoder edges
    epc = -(-ed // NC)                       # 50000
    nb = -(-epc // DB)                       # 13
    in_maps = []
    for c in range(NC):
        tsl = slice(c * TPC, (c + 1) * TPC)
        ia_c = _rep128(_wrap16(ia_cube[tsl].reshape(TPC * 2, CHH * P)))
        smat_c = np.ascontiguousarray(
            smat_all[tsl].transpose(0, 2, 1, 3).reshape(TPC, P, 2 * R * ORD * P))

        e0 = c * epc
        s_pad = np.zeros(nb * DB, np.int64)
        d_pad = np.zeros(nb * DB, np.int64)
        seg = slice(e0, min(e0 + epc, ed))
        ln = seg.stop - seg.start
        s_pad[:ln] = dec_src[seg]
        d_pad[:ln] = dec_dst[seg]
        di = np.concatenate([(s_pad >> 2).reshape(nb, DB),
                             (d_pad >> 2).reshape(nb, DB)], axis=1).astype(np.int16)
        di_c = _rep128(_wrap16(di))          # [128, nb*512]
        mu = (s_pad & 3).astype(np.float32).reshape(nb, 32, P).transpose(0, 2, 1)
        mv = (d_pad & 3).astype(np.float32).reshape(nb, 32, P).transpose(0, 2, 1)
        # [nb, 128, 32] -> repeat 8 -> [nb, 128, 256]
        mu = np.repeat(mu, 8, axis=2)
        mv = np.repeat(mv, 8, axis=2)
        # one-hot select masks per quarter: [nb, 8, 128, 256]
        em = np.stack([(mu == b4).astype(np.float32) for b4 in range(4)]
                      + [(mv == b4).astype(np.float32) for b4 in range(4)], axis=1)
        mk = np.ascontiguousarray(
            em.transpose(2, 0, 1, 3).reshape(P, nb * 8 * 256))
        in_maps.append(dict(
            xA=xA_h, xB=xB_h, WA=WAp, WM=WMp, bias_rep=bias_rep,
            uvb_rep=uvb_rep,
            idx_a=ia_c, smat=smat_c,
            idx_d=di_c, msk_d=mk,
        ))
    dims = dict(CHH=CHH, nb=nb, epc=epc, ed=ed)
    return in_maps, dims


_CACHE = {}


def kernel(**inputs):
    in_maps, dims = _prep(inputs)
    key = (dims["CHH"], dims["nb"])
    nc = _CACHE.get(key)
    if nc is None:
        nc = _build(dims)
        _CACHE[key] = nc
    res = run_bass_kernel_spmd(nc, in_maps, list(range(NC)))
    epc, ed = dims["epc"], dims["ed"]
    out = np.concatenate(
        [res.results[c]["score_out"][:min(epc, ed - c * epc)] for c in range(NC)], axis=0)
    return np.ascontiguousarray(out.astype(np.float32))


if __name__ == "__main__":
    pass

